# revision 31
# baseline (speedup 1.0000x reference)
"""MoE FFN layer (top-2 routing, SwiGLU experts) on 8 Trainium2 NeuronCores.

Sharding: data-parallel over tokens. Each core owns T/8 = 2048 tokens and a
replica of all expert weights. Routing is computed on-device:
  - each core computes gate logits + top-2 + softmax weights for its tokens
  - tiny AllGather of per-token routing info (idx1, idx2, w1, w2) [T, 4]
  - global per-expert capacity positions via matmul-based prefix scans
    (strict-upper-triangular-ones matmuls implement exclusive cumsum)
  - token rows are scattered into per-(core,expert) contiguous groups with one
    indirect DMA per top-k slot (out-of-bounds slot index = skip, which drops
    over-capacity assignments exactly like the reference)
  - per-expert SwiGLU GEMMs over the grouped rows; activations are transposed
    on the PE (identity matmuls) so no DMA-transpose xbar-mode serialization
  - weights are host-prepacked so each W1/W3 f-group and each W2 half-row
    panel is a single contiguous DMA
  - combine: indirect gather of each token's two expert-output rows (bf16) +
    weighted add; output shard is written densely, host concatenates shards.

The per-(core,expert) group capacity CAPL is chosen at call time from a cheap
host-side routing precheck (shapes must be static); the device still computes
all routing itself.  For the reference distribution counts are ~560, CAPL=640.
"""

import math
import os

import numpy as np

import concourse.bass as bass
import concourse.mybir as mybir
from concourse import bacc, tile
from concourse.bass import IndirectOffsetOnAxis
from concourse.bass_utils import run_bass_kernel_spmd

f32 = mybir.dt.float32
bf16 = mybir.dt.bfloat16
i32 = mybir.dt.int32
u32 = mybir.dt.uint32
AF = mybir.ActivationFunctionType
OP = mybir.AluOpType

T, H, F, E = 16384, 1024, 2816, 8
CAP = 5120  # global per-expert capacity = ceil(T * 1.25 * 2 / E)
NCORES = 8
TS = T // NCORES  # tokens per core (2048)
NTT = TS // 128  # token tiles per core (16)
NGT = T // 128  # global token tiles (128)
HT = H // 128  # 8
FT = F // 128  # 22
FG = 2  # f-tiles per W1/W3 weight DMA group
NFG = FT // FG  # 11
BIG = 1.0e6  # "invalid" slot marker, way past any bounds check

LAST_RESULTS = None  # BassKernelResults of the most recent run (for test.py)


CSTW = 520


def _build_consts():
    c = np.zeros((128, CSTW), dtype=np.float32)
    c[:, 0:128] = np.eye(128, dtype=np.float32)  # identity
    iu, ju = np.meshgrid(np.arange(128), np.arange(128), indexing="ij")
    c[:, 128:256] = (iu < ju).astype(np.float32)  # strict upper ones
    c[:, 256:384] = 1.0  # ones
    c[:, 384:392] = np.arange(8, dtype=np.float32)[None, :]  # iota8
    # block-strict: same expert (col%8), strictly earlier token tile (col//8)
    c[:, 392:520] = ((iu % 8 == ju % 8) & (iu // 8 < ju // 8)).astype(np.float32)
    return c


def _scan_all(nc, bps, sb_pool, in_view, ident, ustrict, bstrict, onescol, onesrow):
    """Exclusive prefix-sum over all 8 experts at once. ``in_view`` is
    [128 part, 128 cols] with col = n*8 + e; the scan for each expert e runs
    over its 16 n-columns in (n, partition) order. Returns a PSUM AP
    [128, 128] of per-element exclusive prefix sums."""
    pos = bps.tile([128, 128], f32, name="scan_pos", tag="scan_pos")
    # within-column strict prefix over partitions (all 128 cols at once)
    nc.tensor.matmul(pos[:], lhsT=ustrict, rhs=in_view, start=True, stop=False)
    # per-column sums -> [128 cols, 1]
    csT = bps.tile([128, 1], f32, name="scan_a", tag="scan_a")
    nc.tensor.matmul(csT[:], lhsT=in_view, rhs=onescol, start=True, stop=True)
    csT_sb = sb_pool.tile([128, 1], f32, name="scan_a_sb", tag="scan_a_sb")
    nc.vector.tensor_copy(csT_sb[:], csT[:])
    # exclusive prefix of column sums within each expert's column group
    excl = bps.tile([128, 1], f32, name="scan_b", tag="scan_b")
    nc.tensor.matmul(excl[:], lhsT=bstrict, rhs=csT_sb[:], start=True, stop=True)
    excl_sb = sb_pool.tile([128, 1], f32, name="scan_b_sb", tag="scan_b_sb")
    nc.vector.tensor_copy(excl_sb[:], excl[:])
    # transpose [128,1] -> [1,128]
    exclr = bps.tile([1, 128], f32, name="scan_c", tag="scan_c")
    nc.tensor.matmul(exclr[:], lhsT=excl_sb[:], rhs=ident, start=True, stop=True)
    exclr_sb = sb_pool.tile([1, 128], f32, name="scan_c_sb", tag="scan_c_sb")
    nc.vector.tensor_copy(exclr_sb[:], exclr[:])
    # broadcast the column offsets down all partitions
    nc.tensor.matmul(pos[:], lhsT=onesrow, rhs=exclr_sb[:], start=False, stop=True)
    return pos


def _emit(nc, capl, stage="full", use_bf16=True):
    nsl = E * capl
    nfull = capl // 128  # full 128-row c-tiles per expert group
    rem = capl % 128  # trailing partial c-tile rows (0 or 64)
    ctiles = [(i * 128, 128) for i in range(nfull)]
    if rem:
        ctiles.append((nfull * 128, rem))
    nct = len(ctiles)
    gdt = bf16 if use_bf16 else f32

    def _dump(tc, pool, dram_src, width, dt=f32):
        # copy dram_src[0:TS, 0:width] -> out rows via SBUF
        for i in range(NTT):
            t = pool.tile([128, width], dt, name="dumpt", tag="dumpt")
            nc.sync.dma_start(t[:], dram_src[i * 128 : (i + 1) * 128, 0:width])
            to = pool.tile([128, width], f32, name="dumpto", tag="dumpto")
            nc.vector.tensor_copy(to[:], t[:])
            nc.sync.dma_start(out[i * 128 : (i + 1) * 128, 0:width], to[:])

    xs = nc.dram_tensor("xs", [TS, H], f32, kind="ExternalInput").ap()
    wg = nc.dram_tensor("wg", [H, E], f32, kind="ExternalInput").ap()
    # host-prepacked weights: one contiguous DMA per (e, fg) / (e, hb) panel
    w1 = nc.dram_tensor("w1", [E, NFG, 128, HT * FG * 128], gdt, kind="ExternalInput").ap()
    w3 = nc.dram_tensor("w3", [E, NFG, 128, HT * FG * 128], gdt, kind="ExternalInput").ap()
    w2 = nc.dram_tensor("w2", [E, 2, 128, FT * 512], gdt, kind="ExternalInput").ap()
    cst = nc.dram_tensor("cst", [128, CSTW], f32, kind="ExternalInput").ap()
    premask = nc.dram_tensor("premask", [128, 1], f32, kind="ExternalInput").ap()
    out = nc.dram_tensor("out", [TS, H], f32, kind="ExternalOutput").ap()

    rloc = nc.dram_tensor("rloc", [TS, 2], f32).ap()
    rall = nc.dram_tensor("rall", [T, 2], f32, addr_space="Shared").ap()
    xin = nc.dram_tensor("xin", [nsl, H], gdt).ap()
    eout = nc.dram_tensor("eout", [nsl, H], gdt).ap()

    with tile.TileContext(nc, num_cores=NCORES) as tc:
        with (
            tc.tile_pool(name="persist", bufs=1) as pp,
            tc.tile_pool(name="small", bufs=2) as sp,
        ):
            # ---- constants / static loads ----
            cst_sb = pp.tile([128, CSTW], f32, name="cst", tag="cst")
            nc.sync.dma_start(cst_sb[:], cst)
            ident = cst_sb[:, 0:128]
            ustrict = cst_sb[:, 128:256]
            onescol = cst_sb[:, 256:257]
            onesrow = cst_sb[0:1, 256:384]
            iota8 = cst_sb[:, 384:392]
            bstrict = cst_sb[:, 392:520]

            wg_sb = pp.tile([128, HT * E], f32, name="wg", tag="wg")
            nc.sync.dma_start(
                wg_sb[:].rearrange("p (n e) -> p n e", e=E),
                wg.rearrange("(n p) e -> p n e", p=128),
            )
            pm_sb = pp.tile([128, 1], f32, name="premask", tag="premask")
            nc.sync.dma_start(pm_sb[:], premask)

            # zero only the dummy landing block: slot 0 is the target of all
            # dropped/invalid gather indices, so row 0 must be finite.
            zsb = pp.tile([128, H], gdt, name="zeros", tag="zeros")
            nc.vector.memset(zsb[:], 0.0)
            nc.sync.dma_start(xin[0:128, :], zsb[:])

            ident_g = ident
            if use_bf16:
                identg_sb = pp.tile([128, 128], gdt, name="identg", tag="identg")
                nc.vector.tensor_copy(identg_sb[:], ident)
                ident_g = identg_sb[:]

            # persistent bookkeeping tiles
            jloc = pp.tile([128, NTT * E], f32, name="jloc", tag="jloc")
            oh1 = pp.tile([128, NTT * E], f32, name="oh1", tag="oh1")
            oh2 = pp.tile([128, NTT * E], f32, name="oh2", tag="oh2")
            i1f = pp.tile([128, NTT], f32, name="i1f", tag="i1f")
            i2f = pp.tile([128, NTT], f32, name="i2f", tag="i2f")
            w1l = pp.tile([128, NTT], f32, name="w1l", tag="w1l")
            w2l = pp.tile([128, NTT], f32, name="w2l", tag="w2l")
            vall = pp.tile([128, E * NTT], f32, name="vall", tag="vall")
            lrall = pp.tile([128, E * NTT], f32, name="lrall", tag="lrall")
            offb = pp.tile([128, E], f32, name="offb", tag="offb")
            idxb = pp.tile([128, NTT * 2], f32, name="idxb", tag="idxb")
            idxb3 = idxb[:].rearrange("p (n f) -> p n f", f=2)
            vbb = pp.tile([128, NTT * 2], f32, name="vbb", tag="vbb")
            vbb3 = vbb[:].rearrange("p (n f) -> p n f", f=2)
            sloti = [pp.tile([128, NTT], i32, name=f"slot{k}", tag=f"slot{k}") for k in range(2)]
            gidxi = [pp.tile([128, NTT], i32, name=f"gidx{k}", tag=f"gidx{k}") for k in range(2)]
            wp = [pp.tile([128, NTT], f32, name=f"wp{k}", tag=f"wp{k}") for k in range(2)]

            with tc.tile_pool(name="xspool", bufs=1) as xsp:
                xs_sb = xsp.tile([128, NTT * H], f32, name="xs", tag="xs")
                xs3 = xs_sb[:].rearrange("p (n h) -> p n h", h=H)
                if use_bf16:
                    xsg_sb = xsp.tile([128, NTT * H], gdt, name="xsg", tag="xsg")
                    xsg3 = xsg_sb[:].rearrange("p (n h) -> p n h", h=H)
                else:
                    xsg3 = xs3

                # ================= phase 1: gating =================
                with (
                    tc.tile_pool(name="gps", bufs=3, space="PSUM") as gps,
                    tc.tile_pool(name="gsb", bufs=3) as gsb,
                ):
                    for tt in range(NTT):
                        nc.sync.dma_start(
                            xs3[:, tt, :], xs[tt * 128 : (tt + 1) * 128, :]
                        )
                        if use_bf16:
                            # cast on the otherwise-idle scalar engine
                            nc.scalar.activation(
                                xsg3[:, tt, :], xs3[:, tt, :], AF.Copy
                            )
                        lg = gps.tile([128, E], f32, name="logits", tag="logits")
                        for h in range(HT):
                            tp = gps.tile([128, 128], f32, name="tp", tag="tp")
                            nc.tensor.transpose(
                                tp[:], xs3[:, tt, h * 128 : (h + 1) * 128], ident
                            )
                            xt = gsb.tile([128, 128], f32, name="xT", tag="xT")
                            nc.vector.tensor_copy(xt[:], tp[:])
                            nc.tensor.matmul(
                                lg[:],
                                lhsT=xt[:],
                                rhs=wg_sb[:].rearrange("p (n e) -> p n e", e=E)[
                                    :, h, :
                                ],
                                start=(h == 0),
                                stop=(h == HT - 1),
                            )
                        lgs = gsb.tile([128, E], f32, name="lgs", tag="lgs")
                        nc.vector.tensor_copy(lgs[:], lg[:])
                        v8 = gsb.tile([128, 8], f32, name="v8", tag="v8")
                        nc.vector.max(out=v8[:], in_=lgs[:])
                        i8 = gsb.tile([128, 8], u32, name="i8", tag="i8")
                        nc.vector.max_index(out=i8[:], in_max=v8[:], in_values=lgs[:])
                        nc.vector.tensor_copy(idxb3[:, tt, :], i8[:, 0:2])
                        nc.vector.tensor_copy(vbb3[:, tt, :], v8[:, 0:2])
                        nc.sync.dma_start(
                            rloc[tt * 128 : (tt + 1) * 128, :], idxb3[:, tt, :]
                        )
                    # batched top-2 softmax over all tiles: w1 = 1/(1+d),
                    # w2 = d/(1+d) with d = exp(v2 - v1)
                    dd = gsb.tile([128, NTT], f32, name="dd", tag="dd")
                    nc.vector.tensor_tensor(
                        out=dd[:], in0=vbb3[:, :, 1], in1=vbb3[:, :, 0],
                        op=OP.subtract,
                    )
                    nc.scalar.activation(dd[:], dd[:], AF.Exp)
                    dp1 = gsb.tile([128, NTT], f32, name="dp1", tag="dp1")
                    nc.vector.tensor_scalar_add(dp1[:], dd[:], 1.0)
                    nc.vector.reciprocal(w1l[:], dp1[:])
                    nc.vector.tensor_tensor(
                        out=w2l[:], in0=dd[:], in1=w1l[:], op=OP.mult
                    )

                if stage == "gating":
                    with tc.tile_pool(name="dmp", bufs=2) as dmp:
                        _dump(tc, dmp, rloc, 2)
                    return

                # ============ phase 2+3: allgather + routing matrices ========
                with (
                    tc.tile_pool(name="bps", bufs=1, space="PSUM") as bps,
                    tc.tile_pool(name="ssb", bufs=2) as ssb,
                ):
                    # local routing blocks -> jloc/oh1/oh2/i tiles and the
                    # local scan; all of it reads only this core's routing, so
                    # it is emitted before the collective and overlaps it
                    jloc3 = jloc[:].rearrange("p (n e) -> p n e", e=E)
                    oh13 = oh1[:].rearrange("p (n e) -> p n e", e=E)
                    oh23 = oh2[:].rearrange("p (n e) -> p n e", e=E)
                    nc.vector.tensor_tensor(
                        out=oh13,
                        in0=idxb3[:, :, 0:1].broadcast_to([128, NTT, 8]),
                        in1=iota8.unsqueeze(1).broadcast_to([128, NTT, 8]),
                        op=OP.is_equal,
                    )
                    nc.vector.tensor_tensor(
                        out=oh23,
                        in0=idxb3[:, :, 1:2].broadcast_to([128, NTT, 8]),
                        in1=iota8.unsqueeze(1).broadcast_to([128, NTT, 8]),
                        op=OP.is_equal,
                    )
                    nc.vector.tensor_copy(i1f[:], idxb3[:, :, 0])
                    nc.vector.tensor_copy(i2f[:], idxb3[:, :, 1])
                    nc.vector.tensor_tensor(
                        out=jloc[:], in0=oh1[:], in1=oh2[:], op=OP.add
                    )
                    # batched local scan over all experts (position within
                    # this core's tokens, (n, e) column layout)
                    pos = _scan_all(
                        nc, bps, ssb, jloc[:], ident, ustrict, bstrict,
                        onescol, onesrow,
                    )
                    pos_sb = ssb.tile([128, NTT * E], f32, name="pos_sb", tag="pos_sb")
                    nc.vector.tensor_copy(pos_sb[:], pos[:])

                    nc.gpsimd.collective_compute(
                        "AllGather",
                        OP.bypass,
                        replica_groups=[list(range(NCORES))],
                        ins=[rloc],
                        outs=[rall],
                    )

                    if stage == "allgather":
                        with tc.tile_pool(name="dmp", bufs=2) as dmp:
                            _dump(tc, dmp, rall, 2)
                        return

                    # global per-(tile, expert) membership matrix [128, 128*8]
                    jm = pp.tile([128, NGT * E], f32, name="jm", tag="jm")
                    jm3 = jm[:].rearrange("p (n e) -> p n e", e=E)
                    rsb = ssb.tile([128, NGT * 2], f32, name="rsb", tag="rsb")
                    rsb3 = rsb[:].rearrange("p (n f) -> p n f", f=2)
                    nc.sync.dma_start(
                        rsb3, rall.rearrange("(n p) f -> p n f", p=128)
                    )
                    ohb = ssb.tile([128, NGT * E], f32, name="ohb", tag="ohb")
                    ohb3 = ohb[:].rearrange("p (n e) -> p n e", e=E)
                    nc.vector.tensor_tensor(
                        out=jm3,
                        in0=rsb3[:, :, 0:1].broadcast_to([128, NGT, 8]),
                        in1=iota8.unsqueeze(1).broadcast_to([128, NGT, 8]),
                        op=OP.is_equal,
                    )
                    nc.vector.tensor_tensor(
                        out=ohb3,
                        in0=rsb3[:, :, 1:2].broadcast_to([128, NGT, 8]),
                        in1=iota8.unsqueeze(1).broadcast_to([128, NGT, 8]),
                        op=OP.is_equal,
                    )
                    nc.vector.tensor_tensor(
                        out=jm[:], in0=jm[:], in1=ohb[:], op=OP.add
                    )

                    # per-expert base offsets: count of earlier-core tokens
                    offr = bps.tile([1, E], f32, name="offrow", tag="offrow")
                    for e in range(E):
                        csall = bps.tile([128, 1], f32, name="csall", tag="csall")
                        nc.tensor.matmul(
                            csall[:],
                            lhsT=jm3[:, :, e],
                            rhs=onescol,
                            start=True,
                            stop=True,
                        )
                        csall_sb = ssb.tile([128, 1], f32, name="csall_sb", tag="csall_sb")
                        nc.vector.tensor_copy(csall_sb[:], csall[:])
                        nc.tensor.matmul(
                            offr[0:1, e : e + 1],
                            lhsT=csall_sb[:],
                            rhs=pm_sb[:],
                            start=True,
                            stop=True,
                        )
                    offr_sb = ssb.tile([1, E], f32, name="offr_sb", tag="offr_sb")
                    nc.vector.tensor_copy(offr_sb[:], offr[:])
                    offbp = bps.tile([128, E], f32, name="offbp", tag="offbp")
                    nc.tensor.matmul(
                        offbp[:], lhsT=onesrow, rhs=offr_sb[:], start=True, stop=True
                    )
                    nc.vector.tensor_copy(offb[:], offbp[:])

                    # global position -> validity -> local rank
                    gpos = ssb.tile([128, NTT * E], f32, name="gpos", tag="gpos")
                    nc.vector.tensor_tensor(
                        out=gpos[:].rearrange("p (n e) -> p n e", e=E),
                        in0=pos_sb[:].rearrange("p (n e) -> p n e", e=E),
                        in1=offb[:].unsqueeze(1).broadcast_to([128, NTT, E]),
                        op=OP.add,
                    )
                    nc.vector.scalar_tensor_tensor(
                        out=vall[:],
                        in0=gpos[:],
                        scalar=float(CAP),
                        in1=jloc[:],
                        op0=OP.is_lt,
                        op1=OP.mult,
                    )
                    lr = _scan_all(
                        nc, bps, ssb, vall[:], ident, ustrict, bstrict,
                        onescol, onesrow,
                    )
                    nc.vector.tensor_copy(lrall[:], lr[:])

                    # ---- per-assignment slot / gather-index / weight ----
                    for k, (ikf, ohk, wkl) in enumerate(
                        [(i1f, oh1, w1l), (i2f, oh2, w2l)]
                    ):
                        lrp = ssb.tile([128, NTT], f32, name=f"lrp{k}", tag=f"lrp{k}")
                        vpk = ssb.tile([128, NTT], f32, name=f"vpk{k}", tag=f"vpk{k}")
                        tmp = ssb.tile([128, NTT], f32, name=f"tmp{k}", tag=f"tmp{k}")
                        t128 = ssb.tile([128, NTT * E], f32, name=f"t128_{k}", tag=f"t128_{k}")
                        nc.vector.tensor_tensor(
                            out=t128[:], in0=ohk[:], in1=lrall[:], op=OP.mult
                        )
                        nc.vector.tensor_reduce(
                            out=lrp[:],
                            in_=t128[:].rearrange("p (n e) -> p n e", e=E),
                            axis=mybir.AxisListType.X,
                            op=OP.add,
                        )
                        nc.vector.tensor_tensor(
                            out=t128[:], in0=ohk[:], in1=vall[:], op=OP.mult
                        )
                        nc.vector.tensor_reduce(
                            out=vpk[:],
                            in_=t128[:].rearrange("p (n e) -> p n e", e=E),
                            axis=mybir.AxisListType.X,
                            op=OP.add,
                        )
                        # slot = e*capl + lrank, or >= nsl when invalid
                        slot = ssb.tile([128, NTT], f32, name=f"slotf{k}", tag=f"slotf{k}")
                        nc.vector.scalar_tensor_tensor(
                            out=slot[:],
                            in0=ikf[:],
                            scalar=float(capl),
                            in1=lrp[:],
                            op0=OP.mult,
                            op1=OP.add,
                        )
                        nc.vector.tensor_scalar(
                            out=tmp[:],
                            in0=vpk[:],
                            scalar1=0.5,
                            scalar2=None,
                            op0=OP.is_lt,
                        )
                        nc.vector.scalar_tensor_tensor(
                            out=slot[:],
                            in0=tmp[:],
                            scalar=BIG,
                            in1=slot[:],
                            op0=OP.mult,
                            op1=OP.add,
                        )
                        nc.vector.tensor_copy(sloti[k][:], slot[:])
                        # gather idx = clamped slot, 0 when invalid
                        nc.vector.tensor_scalar_min(tmp[:], lrp[:], float(capl - 1))
                        nc.vector.scalar_tensor_tensor(
                            out=tmp[:],
                            in0=ikf[:],
                            scalar=float(capl),
                            in1=tmp[:],
                            op0=OP.mult,
                            op1=OP.add,
                        )
                        nc.vector.tensor_tensor(
                            out=tmp[:], in0=tmp[:], in1=vpk[:], op=OP.mult
                        )
                        nc.vector.tensor_copy(gidxi[k][:], tmp[:])
                        # combine weight = w_k * valid
                        nc.vector.tensor_tensor(
                            out=wp[k][:], in0=wkl[:], in1=vpk[:], op=OP.mult
                        )

                if stage == "scans":
                    with tc.tile_pool(name="dmp", bufs=2) as dmp:
                        t = dmp.tile([128, 96], f32, name="dumps", tag="dumps")
                        for j, src in enumerate([sloti[0], sloti[1], gidxi[0], gidxi[1], wp[0], wp[1]]):
                            nc.vector.tensor_copy(t[:, j * 16 : (j + 1) * 16], src[:])
                        nc.sync.dma_start(out[0:128, 0:96], t[:])
                    return

                # ============ phase 5: scatter token rows into groups ============
                # The indirect DMA consumes exactly one offset per partition,
                # so this is one call per 128-token tile. Slots are unique per
                # assignment, so the calls write disjoint rows; a critical
                # section with manual completion semaphores lets them stream
                # back-to-back on the gpsimd queue instead of paying a
                # completion round-trip between each pair.
                if os.environ.get("MOE_SCRIT", "1") not in ("", "0"):
                    ssem = nc.alloc_semaphore("scat_sem")
                    with tc.tile_critical():
                        for k in range(2):
                            for tt in range(NTT):
                                nc.gpsimd.indirect_dma_start(
                                    out=xin,
                                    out_offset=IndirectOffsetOnAxis(
                                        ap=sloti[k][:, tt : tt + 1], axis=0
                                    ),
                                    in_=xsg3[:, tt, :],
                                    in_offset=None,
                                    bounds_check=nsl - 1,
                                    oob_is_err=False,
                                ).then_inc(ssem, 16)
                        nc.gpsimd.nop(nofuse=True, hint="scat_wait")._wait_ge(
                            ssem, 2 * NTT * 16
                        )
                else:
                    for k in range(2):
                        for tt in range(NTT):
                            nc.gpsimd.indirect_dma_start(
                                out=xin,
                                out_offset=IndirectOffsetOnAxis(
                                    ap=sloti[k][:, tt : tt + 1], axis=0
                                ),
                                in_=xsg3[:, tt, :],
                                in_offset=None,
                                bounds_check=nsl - 1,
                                oob_is_err=False,
                            )

            if stage == "scatter":
                with tc.tile_pool(name="dmp", bufs=2) as dmp:
                    _dump(tc, dmp, xin, H, dt=gdt)
                return

            # ================= phase 6: expert FFNs =================
            with (
                tc.tile_pool(name="fps_tp", bufs=2, space="PSUM") as fps_tp,
                tc.tile_pool(name="fps_gu", bufs=2, space="PSUM") as fps_gu,
                tc.tile_pool(name="fps_e", bufs=2, space="PSUM") as fps_e,
                tc.tile_pool(name="fsb", bufs=1) as fsb,
                tc.tile_pool(name="fw", bufs=2) as fw,
                tc.tile_pool(name="fio", bufs=2) as fio,
            ):
                cc = [(0, min(512, capl))]
                if capl > 512:
                    cc.append((512, capl - 512))

                def build_actT(e):
                    # build transposed activations on the PE (identity matmuls)
                    actT = fsb.tile([128, HT * capl], gdt, name="actT", tag="actT", bufs=3)
                    actT3 = actT[:].rearrange("p (h c) -> p h c", c=capl)
                    for r0c, rws in ctiles:
                        r0 = e * capl + r0c
                        xi = fio.tile([128, H], gdt, name="xin_sb", tag="xin_sb", bufs=3)
                        nc.sync.dma_start(xi[0:rws, :], xin[r0 : r0 + rws, :])
                        for h in range(HT):
                            tp = fps_tp.tile([128, 128], gdt, name="ffn_tp", tag="ffn_tp")
                            nc.tensor.transpose(
                                tp[:, 0:rws],
                                xi[0:rws, h * 128 : (h + 1) * 128],
                                ident_g[0:rws, 0:rws],
                            )
                            nc.vector.tensor_copy(
                                actT3[:, h, r0c : r0c + rws], tp[:, 0:rws]
                            )
                    return actT3

                actT3_cur = build_actT(0)
                for e in range(E):
                    actT3 = actT3_cur
                    hT = fsb.tile([128, FT * capl], gdt, name="hT", tag="hT", bufs=2)
                    hT3 = hT[:].rearrange("p (f c) -> p f c", c=capl)
                    for fg0 in range(0, FT, FG):
                        fgi = fg0 // FG
                        w1g = fw.tile([128, HT * FG * 128], gdt, name="w1g", tag="w1g")
                        w3g = fw.tile([128, HT * FG * 128], gdt, name="w3g", tag="w3g")
                        w1g3 = w1g[:].rearrange("p (h f) -> p h f", f=FG * 128)
                        w3g3 = w3g[:].rearrange("p (h f) -> p h f", f=FG * 128)
                        nc.sync.dma_start(w1g[:], w1[e, fgi])
                        nc.sync.dma_start(w3g[:], w3[e, fgi])
                        for ft in range(fg0, fg0 + FG):
                            fo = (ft - fg0) * 128
                            ga = [
                                fps_gu.tile([128, w_], f32, name=f"gu{ci}", tag=f"gu{ci}")
                                for ci, (_, w_) in enumerate(cc)
                            ]
                            for h in range(HT):
                                for ci, (c0, w_) in enumerate(cc):
                                    nc.tensor.matmul(
                                        ga[ci][:],
                                        lhsT=w1g3[:, h, fo : fo + 128],
                                        rhs=actT3[:, h, c0 : c0 + w_],
                                        start=(h == 0),
                                        stop=(h == HT - 1),
                                    )
                            # t = silu(g) = g * sigmoid(g)
                            tsl = fio.tile([128, capl], f32, name="tsilu", tag="tsilu")
                            for ci, (c0, w_) in enumerate(cc):
                                nc.scalar.activation(
                                    tsl[:, c0 : c0 + w_], ga[ci][:], AF.Sigmoid
                                )
                                nc.vector.tensor_tensor(
                                    out=tsl[:, c0 : c0 + w_],
                                    in0=tsl[:, c0 : c0 + w_],
                                    in1=ga[ci][:],
                                    op=OP.mult,
                                )
                            # u = x @ W3 (reuse psum slots)
                            ua = [
                                fps_gu.tile([128, w_], f32, name=f"gu{ci}", tag=f"gu{ci}")
                                for ci, (_, w_) in enumerate(cc)
                            ]
                            for h in range(HT):
                                for ci, (c0, w_) in enumerate(cc):
                                    nc.tensor.matmul(
                                        ua[ci][:],
                                        lhsT=w3g3[:, h, fo : fo + 128],
                                        rhs=actT3[:, h, c0 : c0 + w_],
                                        start=(h == 0),
                                        stop=(h == HT - 1),
                                    )
                            # hT = silu(g) * u
                            for ci, (c0, w_) in enumerate(cc):
                                nc.vector.tensor_tensor(
                                    out=hT3[:, ft, c0 : c0 + w_],
                                    in0=tsl[:, c0 : c0 + w_],
                                    in1=ua[ci][:],
                                    op=OP.mult,
                                )
                    # emit the next expert's activation-transpose build here so
                    # its DVE copies drain underneath pass 2's matmul stream
                    if e + 1 < E:
                        actT3_cur = build_actT(e + 1)
                    # pass 2: eout = hT.T @ W2
                    for hb in range(2):
                        w2r = fsb.tile([128, FT * 512], gdt, name="w2row", tag="w2row", bufs=2)
                        w2r3 = w2r[:].rearrange("p (f x) -> p f x", x=512)
                        nc.sync.dma_start(w2r[:], w2[e, hb])
                        eo = fio.tile([128, nct * 512], gdt, name="eo_sb", tag="eo_sb")
                        eo3 = eo[:].rearrange("p (n x) -> p n x", x=512)
                        for ci, (r0c, rws) in enumerate(ctiles):
                            eps = fps_e.tile([128, 512], f32, name="eps", tag="eps")
                            for ft in range(FT):
                                nc.tensor.matmul(
                                    eps[0:rws, :],
                                    lhsT=hT3[:, ft, r0c : r0c + rws],
                                    rhs=w2r3[:, ft, :],
                                    start=(ft == 0),
                                    stop=(ft == FT - 1),
                                )
                            nc.vector.tensor_copy(eo3[0:rws, ci, :], eps[0:rws, :])
                        # batched store per (e, hb): full tiles in one
                        # rearranged DMA, trailing partial tile separately
                        nc.sync.dma_start(
                            eout[e * capl : e * capl + nfull * 128,
                                 hb * 512 : (hb + 1) * 512]
                            .rearrange("(n p) x -> p n x", p=128),
                            eo3[:, 0:nfull, :],
                        )
                        if rem:
                            nc.sync.dma_start(
                                eout[e * capl + nfull * 128 : (e + 1) * capl,
                                     hb * 512 : (hb + 1) * 512],
                                eo3[0:rem, nfull, :],
                            )

                if stage == "ffn":
                    with tc.tile_pool(name="dmp2", bufs=2) as dmp2:
                        _dump(tc, dmp2, eout, H, dt=gdt)
                    return

                # ================= phase 7: combine =================
                for tt in range(NTT):
                    r1 = fio.tile([128, H], gdt, name="r1", tag="r1")
                    nc.gpsimd.indirect_dma_start(
                        out=r1[:],
                        out_offset=None,
                        in_=eout,
                        in_offset=IndirectOffsetOnAxis(
                            ap=gidxi[0][:, tt : tt + 1], axis=0
                        ),
                    )
                    r2 = fio.tile([128, H], gdt, name="r2", tag="r2")
                    nc.gpsimd.indirect_dma_start(
                        out=r2[:],
                        out_offset=None,
                        in_=eout,
                        in_offset=IndirectOffsetOnAxis(
                            ap=gidxi[1][:, tt : tt + 1], axis=0
                        ),
                    )
                    ot = fio.tile([128, H], f32, name="ot", tag="ot")
                    nc.vector.tensor_scalar(
                        out=ot[:],
                        in0=r1[:],
                        scalar1=wp[0][:, tt : tt + 1],
                        scalar2=None,
                        op0=OP.mult,
                    )
                    nc.vector.scalar_tensor_tensor(
                        out=ot[:],
                        in0=r2[:],
                        scalar=wp[1][:, tt : tt + 1],
                        in1=ot[:],
                        op0=OP.mult,
                        op1=OP.add,
                    )
                    nc.sync.dma_start(out[tt * 128 : (tt + 1) * 128, :], ot[:])

    return nc


_LDW_PATCHED = False


def _enable_ldw_opt():
    """Swap the hardcoded --enable-ldw-opt=false walrus flag to true: every
    fp32 matmul otherwise pays an unoverlapped LDWEIGHTS (~40% PE time)."""
    global _LDW_PATCHED
    if _LDW_PATCHED:
        return
    from concourse import bass_utils as _bu

    _orig = _bu.run_command

    def _patched(argv, **kw):
        argv = [
            a.replace("--enable-ldw-opt=false", "--enable-ldw-opt=true")
            if isinstance(a, str)
            else a
            for a in argv
        ]
        return _orig(argv, **kw)

    _bu.run_command = _patched
    _LDW_PATCHED = True


_NC_CACHE = {}


def _get_nc(capl, stage="full", use_bf16=True):
    # NOTE: --enable-ldw-opt=true crashes walrus codegen (visitInstLdweights,
    # CoreV3GenImpl.cpp:694) on this kernel — keep it off.
    if os.environ.get("MOE_LDW_OPT", "0") not in ("", "0"):
        _enable_ldw_opt()
    key = (capl, stage, use_bf16)
    if key not in _NC_CACHE:
        nc = bacc.Bacc("TRN2", debug=False, num_devices=NCORES)
        _emit(nc, capl, stage, use_bf16)
        nc.compile()
        _NC_CACHE[key] = nc
    return _NC_CACHE[key]


def _host_max_local_count(x, Wg):
    """Cheap host routing replica: max kept-assignments per (core, expert)."""
    logits = x.astype(np.float32) @ Wg.astype(np.float32)
    i1 = np.argmax(logits, axis=1)
    m = logits.copy()
    m[np.arange(T), i1] = -np.inf
    i2 = np.argmax(m, axis=1)
    routed = np.zeros((T, E), dtype=np.int64)
    routed[np.arange(T), i1] = 1
    routed[np.arange(T), i2] += 1
    pos = np.cumsum(routed, axis=0) - routed
    keep = routed * (pos < CAP)
    counts = keep.reshape(NCORES, TS, E).sum(axis=1)
    return int(counts.max())


def _install_ntff_hook():
    """Best-effort registration of the axon NTFF profiling hook (for tracing)."""
    import sys
    import types

    if "antenv.axon_hooks" in sys.modules:
        return
    try:
        mod = types.ModuleType("antenv.axon_hooks")
        hook = [None]
        mod.set_axon_ntff_profile_hook = lambda h: hook.__setitem__(0, h)
        mod.get_axon_ntff_profile_hook = lambda: hook[0]
        from trn_agent_boot.trn_boot import _ntff_profile_via_ctypes

        mod.set_axon_ntff_profile_hook(
            _ntff_profile_via_ctypes("/opt/axon/libaxon_pjrt.so")
        )
        sys.modules["antenv.axon_hooks"] = mod
    except Exception:
        pass


def kernel(x, Wg, W1, W3, W2):
    global LAST_RESULTS
    x = np.ascontiguousarray(np.asarray(x, dtype=np.float32))
    Wg = np.ascontiguousarray(np.asarray(Wg, dtype=np.float32))
    W1 = np.asarray(W1, dtype=np.float32)
    W3 = np.asarray(W3, dtype=np.float32)
    W2 = np.asarray(W2, dtype=np.float32)

    # static per-(core, expert) group capacity with safety margin (device
    # routing could diverge from this host estimate only on exact ties)
    maxc = _host_max_local_count(x, Wg)
    capl = max(128, int(math.ceil((maxc + 16) / 64.0) * 64))

    use_bf16 = os.environ.get("MOE_GDT", "bf16") != "f32"
    nc = _get_nc(capl, os.environ.get("MOE_STAGE", "full"), use_bf16)
    cst = _build_consts()
    wdt = np.float32
    if use_bf16:
        import ml_dtypes

        wdt = ml_dtypes.bfloat16
    # host-prepack the weight panels so each on-device load is one
    # contiguous-per-partition DMA:
    #   w1p[e, fg, p, (ht, fo)] = W1[e, ht*128+p, fg*256+fo]
    #   w2p[e, hb, p, (ft, x)]  = W2[e, ft*128+p, hb*512+x]
    w1p = np.ascontiguousarray(
        W1.reshape(E, HT, 128, NFG, FG * 128).transpose(0, 3, 2, 1, 4)
        .reshape(E, NFG, 128, HT * FG * 128).astype(wdt)
    )
    w3p = np.ascontiguousarray(
        W3.reshape(E, HT, 128, NFG, FG * 128).transpose(0, 3, 2, 1, 4)
        .reshape(E, NFG, 128, HT * FG * 128).astype(wdt)
    )
    w2p = np.ascontiguousarray(
        W2.reshape(E, FT, 128, 2, 512).transpose(0, 3, 2, 1, 4)
        .reshape(E, 2, 128, FT * 512).astype(wdt)
    )
    in_maps = []
    for c in range(NCORES):
        pm = (np.arange(128) < c * NTT).astype(np.float32)[:, None]
        in_maps.append(
            {
                "xs": x[c * TS : (c + 1) * TS],
                "wg": Wg,
                "w1": w1p,
                "w3": w3p,
                "w2": w2p,
                "cst": cst,
                "premask": np.ascontiguousarray(pm),
            }
        )

    trace = os.environ.get("BASS_TRACE", "") not in ("", "0", "false", "False")
    if trace:
        _install_ntff_hook()
    res = run_bass_kernel_spmd(nc, in_maps, list(range(NCORES)), trace=trace)
    LAST_RESULTS = res
    return np.concatenate([res.results[c]["out"] for c in range(NCORES)], axis=0)


# revision 37
# speedup vs baseline: 1.0016x; 1.0016x over previous
"""MoE FFN layer (top-2 routing, SwiGLU experts) on 8 Trainium2 NeuronCores.

Sharding: data-parallel over tokens. Each core owns T/8 = 2048 tokens and a
replica of all expert weights. Routing is computed on-device:
  - each core computes gate logits + top-2 + softmax weights for its tokens
  - tiny AllGather of per-token routing info (idx1, idx2, w1, w2) [T, 4]
  - global per-expert capacity positions via matmul-based prefix scans
    (strict-upper-triangular-ones matmuls implement exclusive cumsum)
  - token rows are scattered into per-(core,expert) contiguous groups with one
    indirect DMA per top-k slot (out-of-bounds slot index = skip, which drops
    over-capacity assignments exactly like the reference)
  - per-expert SwiGLU GEMMs over the grouped rows; activations are transposed
    on the PE (identity matmuls) so no DMA-transpose xbar-mode serialization
  - weights are host-prepacked so each W1/W3 f-group and each W2 half-row
    panel is a single contiguous DMA
  - combine: indirect gather of each token's two expert-output rows (bf16) +
    weighted add; output shard is written densely, host concatenates shards.

The per-(core,expert) group capacity CAPL is chosen at call time from a cheap
host-side routing precheck (shapes must be static); the device still computes
all routing itself.  For the reference distribution counts are ~560, CAPL=640.
"""

import math
import os

import numpy as np

import concourse.bass as bass
import concourse.mybir as mybir
from concourse import bacc, tile
from concourse.bass import IndirectOffsetOnAxis
from concourse.bass_utils import run_bass_kernel_spmd

f32 = mybir.dt.float32
bf16 = mybir.dt.bfloat16
i32 = mybir.dt.int32
u32 = mybir.dt.uint32
AF = mybir.ActivationFunctionType
OP = mybir.AluOpType

T, H, F, E = 16384, 1024, 2816, 8
CAP = 5120  # global per-expert capacity = ceil(T * 1.25 * 2 / E)
NCORES = 8
TS = T // NCORES  # tokens per core (2048)
NTT = TS // 128  # token tiles per core (16)
NGT = T // 128  # global token tiles (128)
HT = H // 128  # 8
FT = F // 128  # 22
FG = 2  # f-tiles per W1/W3 weight DMA group
NFG = FT // FG  # 11
BIG = 1.0e6  # "invalid" slot marker, way past any bounds check

LAST_RESULTS = None  # BassKernelResults of the most recent run (for test.py)


CSTW = 520


def _build_consts():
    c = np.zeros((128, CSTW), dtype=np.float32)
    c[:, 0:128] = np.eye(128, dtype=np.float32)  # identity
    iu, ju = np.meshgrid(np.arange(128), np.arange(128), indexing="ij")
    c[:, 128:256] = (iu < ju).astype(np.float32)  # strict upper ones
    c[:, 256:384] = 1.0  # ones
    c[:, 384:392] = np.arange(8, dtype=np.float32)[None, :]  # iota8
    # block-strict: same expert (col%8), strictly earlier token tile (col//8)
    c[:, 392:520] = ((iu % 8 == ju % 8) & (iu // 8 < ju // 8)).astype(np.float32)
    return c


def _scan_all(nc, bps, sb_pool, in_view, ident, ustrict, bstrict, onescol, onesrow):
    """Exclusive prefix-sum over all 8 experts at once. ``in_view`` is
    [128 part, 128 cols] with col = n*8 + e; the scan for each expert e runs
    over its 16 n-columns in (n, partition) order. Returns a PSUM AP
    [128, 128] of per-element exclusive prefix sums."""
    pos = bps.tile([128, 128], f32, name="scan_pos", tag="scan_pos")
    # within-column strict prefix over partitions (all 128 cols at once)
    nc.tensor.matmul(pos[:], lhsT=ustrict, rhs=in_view, start=True, stop=False)
    # per-column sums -> [128 cols, 1]
    csT = bps.tile([128, 1], f32, name="scan_a", tag="scan_a")
    nc.tensor.matmul(csT[:], lhsT=in_view, rhs=onescol, start=True, stop=True)
    csT_sb = sb_pool.tile([128, 1], f32, name="scan_a_sb", tag="scan_a_sb")
    nc.vector.tensor_copy(csT_sb[:], csT[:])
    # exclusive prefix of column sums within each expert's column group
    excl = bps.tile([128, 1], f32, name="scan_b", tag="scan_b")
    nc.tensor.matmul(excl[:], lhsT=bstrict, rhs=csT_sb[:], start=True, stop=True)
    excl_sb = sb_pool.tile([128, 1], f32, name="scan_b_sb", tag="scan_b_sb")
    nc.vector.tensor_copy(excl_sb[:], excl[:])
    # transpose [128,1] -> [1,128]
    exclr = bps.tile([1, 128], f32, name="scan_c", tag="scan_c")
    nc.tensor.matmul(exclr[:], lhsT=excl_sb[:], rhs=ident, start=True, stop=True)
    exclr_sb = sb_pool.tile([1, 128], f32, name="scan_c_sb", tag="scan_c_sb")
    nc.vector.tensor_copy(exclr_sb[:], exclr[:])
    # broadcast the column offsets down all partitions
    nc.tensor.matmul(pos[:], lhsT=onesrow, rhs=exclr_sb[:], start=False, stop=True)
    return pos


def _emit(nc, capl, stage="full", use_bf16=True):
    nsl = E * capl
    nfull = capl // 128  # full 128-row c-tiles per expert group
    rem = capl % 128  # trailing partial c-tile rows (0 or 64)
    ctiles = [(i * 128, 128) for i in range(nfull)]
    if rem:
        ctiles.append((nfull * 128, rem))
    nct = len(ctiles)
    gdt = bf16 if use_bf16 else f32

    def _dump(tc, pool, dram_src, width, dt=f32):
        # copy dram_src[0:TS, 0:width] -> out rows via SBUF
        for i in range(NTT):
            t = pool.tile([128, width], dt, name="dumpt", tag="dumpt")
            nc.sync.dma_start(t[:], dram_src[i * 128 : (i + 1) * 128, 0:width])
            to = pool.tile([128, width], f32, name="dumpto", tag="dumpto")
            nc.vector.tensor_copy(to[:], t[:])
            nc.sync.dma_start(out[i * 128 : (i + 1) * 128, 0:width], to[:])

    xs = nc.dram_tensor("xs", [TS, H], f32, kind="ExternalInput").ap()
    wg = nc.dram_tensor("wg", [H, E], f32, kind="ExternalInput").ap()
    # host-prepacked weights: one contiguous DMA per (e, fg) / (e, hb) panel
    w1 = nc.dram_tensor("w1", [E, NFG, 128, HT * FG * 128], gdt, kind="ExternalInput").ap()
    w3 = nc.dram_tensor("w3", [E, NFG, 128, HT * FG * 128], gdt, kind="ExternalInput").ap()
    w2 = nc.dram_tensor("w2", [E, 2, 128, FT * 512], gdt, kind="ExternalInput").ap()
    cst = nc.dram_tensor("cst", [128, CSTW], f32, kind="ExternalInput").ap()
    premask = nc.dram_tensor("premask", [128, 1], f32, kind="ExternalInput").ap()
    out = nc.dram_tensor("out", [TS, H], f32, kind="ExternalOutput").ap()

    rloc = nc.dram_tensor("rloc", [TS, 2], f32).ap()
    # two-half AllGather outputs: half h holds all cores' token tiles
    # [h*8, h*8+8) in (core, tile, token) order
    rallA = nc.dram_tensor("rallA", [T // 2, 2], f32, addr_space="Shared").ap()
    rallB = nc.dram_tensor("rallB", [T // 2, 2], f32, addr_space="Shared").ap()
    xin = nc.dram_tensor("xin", [nsl, H], gdt).ap()
    eout = nc.dram_tensor("eout", [nsl, H], gdt).ap()

    with tile.TileContext(nc, num_cores=NCORES) as tc:
        with (
            tc.tile_pool(name="persist", bufs=1) as pp,
            tc.tile_pool(name="small", bufs=2) as sp,
        ):
            # ---- constants / static loads ----
            cst_sb = pp.tile([128, CSTW], f32, name="cst", tag="cst")
            nc.sync.dma_start(cst_sb[:], cst)
            ident = cst_sb[:, 0:128]
            ustrict = cst_sb[:, 128:256]
            onescol = cst_sb[:, 256:257]
            onesrow = cst_sb[0:1, 256:384]
            iota8 = cst_sb[:, 384:392]
            bstrict = cst_sb[:, 392:520]

            wg_sb = pp.tile([128, HT * E], f32, name="wg", tag="wg")
            nc.sync.dma_start(
                wg_sb[:].rearrange("p (n e) -> p n e", e=E),
                wg.rearrange("(n p) e -> p n e", p=128),
            )
            pm_sb = pp.tile([128, 1], f32, name="premask", tag="premask")
            nc.sync.dma_start(pm_sb[:], premask)

            # zero only the dummy landing block: slot 0 is the target of all
            # dropped/invalid gather indices, so row 0 must be finite.
            zsb = pp.tile([128, H], gdt, name="zeros", tag="zeros")
            nc.vector.memset(zsb[:], 0.0)
            nc.sync.dma_start(xin[0:128, :], zsb[:])

            ident_g = ident
            if use_bf16:
                identg_sb = pp.tile([128, 128], gdt, name="identg", tag="identg")
                nc.vector.tensor_copy(identg_sb[:], ident)
                ident_g = identg_sb[:]

            # persistent bookkeeping tiles
            jloc = pp.tile([128, NTT * E], f32, name="jloc", tag="jloc")
            oh1 = pp.tile([128, NTT * E], f32, name="oh1", tag="oh1")
            oh2 = pp.tile([128, NTT * E], f32, name="oh2", tag="oh2")
            i1f = pp.tile([128, NTT], f32, name="i1f", tag="i1f")
            i2f = pp.tile([128, NTT], f32, name="i2f", tag="i2f")
            w1l = pp.tile([128, NTT], f32, name="w1l", tag="w1l")
            w2l = pp.tile([128, NTT], f32, name="w2l", tag="w2l")
            vall = pp.tile([128, E * NTT], f32, name="vall", tag="vall")
            lrall = pp.tile([128, E * NTT], f32, name="lrall", tag="lrall")
            offb = pp.tile([128, E], f32, name="offb", tag="offb")
            idxb = pp.tile([128, NTT * 2], f32, name="idxb", tag="idxb")
            idxb3 = idxb[:].rearrange("p (n f) -> p n f", f=2)
            vbb = pp.tile([128, NTT * 2], f32, name="vbb", tag="vbb")
            vbb3 = vbb[:].rearrange("p (n f) -> p n f", f=2)
            sloti = [pp.tile([128, NTT], i32, name=f"slot{k}", tag=f"slot{k}") for k in range(2)]
            gidxi = [pp.tile([128, NTT], i32, name=f"gidx{k}", tag=f"gidx{k}") for k in range(2)]
            wp = [pp.tile([128, NTT], f32, name=f"wp{k}", tag=f"wp{k}") for k in range(2)]

            with tc.tile_pool(name="xspool", bufs=1) as xsp:
                xs_sb = xsp.tile([128, NTT * H], f32, name="xs", tag="xs")
                xs3 = xs_sb[:].rearrange("p (n h) -> p n h", h=H)
                if use_bf16:
                    xsg_sb = xsp.tile([128, NTT * H], gdt, name="xsg", tag="xsg")
                    xsg3 = xsg_sb[:].rearrange("p (n h) -> p n h", h=H)
                else:
                    xsg3 = xs3

                # ================= phase 1: gating =================
                with (
                    tc.tile_pool(name="gps", bufs=3, space="PSUM") as gps,
                    tc.tile_pool(name="gsb", bufs=3) as gsb,
                ):
                    for tt in range(NTT):
                        nc.sync.dma_start(
                            xs3[:, tt, :], xs[tt * 128 : (tt + 1) * 128, :]
                        )
                        if use_bf16:
                            # cast on the otherwise-idle scalar engine
                            nc.scalar.activation(
                                xsg3[:, tt, :], xs3[:, tt, :], AF.Copy
                            )
                        lg = gps.tile([128, E], f32, name="logits", tag="logits")
                        for hg in range(0, HT, 4):
                            # 4 transposes share one PSUM bank tile -> one
                            # wide copy, coarse PE<->DVE ping-pong
                            tpg = gps.tile([128, 512], f32, name="tpg", tag="tpg")
                            for j in range(4):
                                h = hg + j
                                nc.tensor.transpose(
                                    tpg[:, j * 128 : (j + 1) * 128],
                                    xs3[:, tt, h * 128 : (h + 1) * 128],
                                    ident,
                                )
                            xtg = gsb.tile([128, 512], f32, name="xtg", tag="xtg")
                            nc.vector.tensor_copy(xtg[:], tpg[:])
                            for j in range(4):
                                h = hg + j
                                nc.tensor.matmul(
                                    lg[:],
                                    lhsT=xtg[:, j * 128 : (j + 1) * 128],
                                    rhs=wg_sb[:].rearrange("p (n e) -> p n e", e=E)[
                                        :, h, :
                                    ],
                                    start=(h == 0),
                                    stop=(h == HT - 1),
                                )
                        lgs = gsb.tile([128, E], f32, name="lgs", tag="lgs")
                        nc.vector.tensor_copy(lgs[:], lg[:])
                        v8 = gsb.tile([128, 8], f32, name="v8", tag="v8")
                        nc.vector.max(out=v8[:], in_=lgs[:])
                        i8 = gsb.tile([128, 8], u32, name="i8", tag="i8")
                        nc.vector.max_index(out=i8[:], in_max=v8[:], in_values=lgs[:])
                        nc.vector.tensor_copy(idxb3[:, tt, :], i8[:, 0:2])
                        nc.vector.tensor_copy(vbb3[:, tt, :], v8[:, 0:2])
                        nc.sync.dma_start(
                            rloc[tt * 128 : (tt + 1) * 128, :], idxb3[:, tt, :]
                        )
                        if tt == NTT // 2 - 1:
                            # first-half routing is final: allgather it while
                            # the second half of gating runs
                            nc.gpsimd.collective_compute(
                                "AllGather",
                                OP.bypass,
                                replica_groups=[list(range(NCORES))],
                                ins=[rloc[0 : TS // 2, :]],
                                outs=[rallA],
                            )
                    # batched top-2 softmax over all tiles: w1 = 1/(1+d),
                    # w2 = d/(1+d) with d = exp(v2 - v1)
                    dd = gsb.tile([128, NTT], f32, name="dd", tag="dd")
                    nc.vector.tensor_tensor(
                        out=dd[:], in0=vbb3[:, :, 1], in1=vbb3[:, :, 0],
                        op=OP.subtract,
                    )
                    nc.scalar.activation(dd[:], dd[:], AF.Exp)
                    dp1 = gsb.tile([128, NTT], f32, name="dp1", tag="dp1")
                    nc.vector.tensor_scalar_add(dp1[:], dd[:], 1.0)
                    nc.vector.reciprocal(w1l[:], dp1[:])
                    nc.vector.tensor_tensor(
                        out=w2l[:], in0=dd[:], in1=w1l[:], op=OP.mult
                    )

                if stage == "gating":
                    with tc.tile_pool(name="dmp", bufs=2) as dmp:
                        _dump(tc, dmp, rloc, 2)
                    return

                # ============ phase 2+3: allgather + routing matrices ========
                with (
                    tc.tile_pool(name="bps", bufs=1, space="PSUM") as bps,
                    tc.tile_pool(name="ssb", bufs=2) as ssb,
                ):
                    # local routing blocks -> jloc/oh1/oh2/i tiles and the
                    # local scan; all of it reads only this core's routing, so
                    # it is emitted before the collective and overlaps it
                    jloc3 = jloc[:].rearrange("p (n e) -> p n e", e=E)
                    oh13 = oh1[:].rearrange("p (n e) -> p n e", e=E)
                    oh23 = oh2[:].rearrange("p (n e) -> p n e", e=E)
                    nc.vector.tensor_tensor(
                        out=oh13,
                        in0=idxb3[:, :, 0:1].broadcast_to([128, NTT, 8]),
                        in1=iota8.unsqueeze(1).broadcast_to([128, NTT, 8]),
                        op=OP.is_equal,
                    )
                    nc.vector.tensor_tensor(
                        out=oh23,
                        in0=idxb3[:, :, 1:2].broadcast_to([128, NTT, 8]),
                        in1=iota8.unsqueeze(1).broadcast_to([128, NTT, 8]),
                        op=OP.is_equal,
                    )
                    nc.vector.tensor_copy(i1f[:], idxb3[:, :, 0])
                    nc.vector.tensor_copy(i2f[:], idxb3[:, :, 1])
                    nc.vector.tensor_tensor(
                        out=jloc[:], in0=oh1[:], in1=oh2[:], op=OP.add
                    )
                    # batched local scan over all experts (position within
                    # this core's tokens, (n, e) column layout)
                    pos = _scan_all(
                        nc, bps, ssb, jloc[:], ident, ustrict, bstrict,
                        onescol, onesrow,
                    )
                    pos_sb = ssb.tile([128, NTT * E], f32, name="pos_sb", tag="pos_sb")
                    nc.vector.tensor_copy(pos_sb[:], pos[:])

                    nc.gpsimd.collective_compute(
                        "AllGather",
                        OP.bypass,
                        replica_groups=[list(range(NCORES))],
                        ins=[rloc[TS // 2 : TS, :]],
                        outs=[rallB],
                    )

                    # global per-(tile, expert) membership matrix [128, 128*8].
                    # Column groups are ordered (half, core, tile-within-half)
                    # so each AllGather half lands contiguously; the premask
                    # (earlier-core weighting) uses the same ordering.
                    jm = pp.tile([128, NGT * E], f32, name="jm", tag="jm")
                    jm3 = jm[:].rearrange("p (n e) -> p n e", e=E)
                    rsb = ssb.tile([128, NGT * 2], f32, name="rsb", tag="rsb")
                    rsb3 = rsb[:].rearrange("p (n f) -> p n f", f=2)
                    nc.sync.dma_start(
                        rsb3[:, 0 : NGT // 2, :],
                        rallA.rearrange("(g p) f -> p g f", p=128),
                    )
                    nc.sync.dma_start(
                        rsb3[:, NGT // 2 : NGT, :],
                        rallB.rearrange("(g p) f -> p g f", p=128),
                    )
                    ohb = ssb.tile([128, NGT * E], f32, name="ohb", tag="ohb")
                    ohb3 = ohb[:].rearrange("p (n e) -> p n e", e=E)
                    nc.vector.tensor_tensor(
                        out=jm3,
                        in0=rsb3[:, :, 0:1].broadcast_to([128, NGT, 8]),
                        in1=iota8.unsqueeze(1).broadcast_to([128, NGT, 8]),
                        op=OP.is_equal,
                    )
                    nc.vector.tensor_tensor(
                        out=ohb3,
                        in0=rsb3[:, :, 1:2].broadcast_to([128, NGT, 8]),
                        in1=iota8.unsqueeze(1).broadcast_to([128, NGT, 8]),
                        op=OP.is_equal,
                    )
                    nc.vector.tensor_tensor(
                        out=jm[:], in0=jm[:], in1=ohb[:], op=OP.add
                    )

                    # per-expert base offsets: count of earlier-core tokens
                    offr = bps.tile([1, E], f32, name="offrow", tag="offrow")
                    for e in range(E):
                        csall = bps.tile([128, 1], f32, name="csall", tag="csall")
                        nc.tensor.matmul(
                            csall[:],
                            lhsT=jm3[:, :, e],
                            rhs=onescol,
                            start=True,
                            stop=True,
                        )
                        csall_sb = ssb.tile([128, 1], f32, name="csall_sb", tag="csall_sb")
                        nc.vector.tensor_copy(csall_sb[:], csall[:])
                        nc.tensor.matmul(
                            offr[0:1, e : e + 1],
                            lhsT=csall_sb[:],
                            rhs=pm_sb[:],
                            start=True,
                            stop=True,
                        )
                    offr_sb = ssb.tile([1, E], f32, name="offr_sb", tag="offr_sb")
                    nc.vector.tensor_copy(offr_sb[:], offr[:])
                    offbp = bps.tile([128, E], f32, name="offbp", tag="offbp")
                    nc.tensor.matmul(
                        offbp[:], lhsT=onesrow, rhs=offr_sb[:], start=True, stop=True
                    )
                    nc.vector.tensor_copy(offb[:], offbp[:])

                    # global position -> validity -> local rank
                    gpos = ssb.tile([128, NTT * E], f32, name="gpos", tag="gpos")
                    nc.vector.tensor_tensor(
                        out=gpos[:].rearrange("p (n e) -> p n e", e=E),
                        in0=pos_sb[:].rearrange("p (n e) -> p n e", e=E),
                        in1=offb[:].unsqueeze(1).broadcast_to([128, NTT, E]),
                        op=OP.add,
                    )
                    nc.vector.scalar_tensor_tensor(
                        out=vall[:],
                        in0=gpos[:],
                        scalar=float(CAP),
                        in1=jloc[:],
                        op0=OP.is_lt,
                        op1=OP.mult,
                    )
                    lr = _scan_all(
                        nc, bps, ssb, vall[:], ident, ustrict, bstrict,
                        onescol, onesrow,
                    )
                    nc.vector.tensor_copy(lrall[:], lr[:])

                    # ---- per-assignment slot / gather-index / weight ----
                    for k, (ikf, ohk, wkl) in enumerate(
                        [(i1f, oh1, w1l), (i2f, oh2, w2l)]
                    ):
                        lrp = ssb.tile([128, NTT], f32, name=f"lrp{k}", tag=f"lrp{k}")
                        vpk = ssb.tile([128, NTT], f32, name=f"vpk{k}", tag=f"vpk{k}")
                        tmp = ssb.tile([128, NTT], f32, name=f"tmp{k}", tag=f"tmp{k}")
                        t128 = ssb.tile([128, NTT * E], f32, name=f"t128_{k}", tag=f"t128_{k}")
                        nc.vector.tensor_tensor(
                            out=t128[:], in0=ohk[:], in1=lrall[:], op=OP.mult
                        )
                        nc.vector.tensor_reduce(
                            out=lrp[:],
                            in_=t128[:].rearrange("p (n e) -> p n e", e=E),
                            axis=mybir.AxisListType.X,
                            op=OP.add,
                        )
                        nc.vector.tensor_tensor(
                            out=t128[:], in0=ohk[:], in1=vall[:], op=OP.mult
                        )
                        nc.vector.tensor_reduce(
                            out=vpk[:],
                            in_=t128[:].rearrange("p (n e) -> p n e", e=E),
                            axis=mybir.AxisListType.X,
                            op=OP.add,
                        )
                        # slot = e*capl + lrank, or >= nsl when invalid
                        slot = ssb.tile([128, NTT], f32, name=f"slotf{k}", tag=f"slotf{k}")
                        nc.vector.scalar_tensor_tensor(
                            out=slot[:],
                            in0=ikf[:],
                            scalar=float(capl),
                            in1=lrp[:],
                            op0=OP.mult,
                            op1=OP.add,
                        )
                        nc.vector.tensor_scalar(
                            out=tmp[:],
                            in0=vpk[:],
                            scalar1=0.5,
                            scalar2=None,
                            op0=OP.is_lt,
                        )
                        nc.vector.scalar_tensor_tensor(
                            out=slot[:],
                            in0=tmp[:],
                            scalar=BIG,
                            in1=slot[:],
                            op0=OP.mult,
                            op1=OP.add,
                        )
                        nc.vector.tensor_copy(sloti[k][:], slot[:])
                        # gather idx = clamped slot, 0 when invalid
                        nc.vector.tensor_scalar_min(tmp[:], lrp[:], float(capl - 1))
                        nc.vector.scalar_tensor_tensor(
                            out=tmp[:],
                            in0=ikf[:],
                            scalar=float(capl),
                            in1=tmp[:],
                            op0=OP.mult,
                            op1=OP.add,
                        )
                        nc.vector.tensor_tensor(
                            out=tmp[:], in0=tmp[:], in1=vpk[:], op=OP.mult
                        )
                        nc.vector.tensor_copy(gidxi[k][:], tmp[:])
                        # combine weight = w_k * valid
                        nc.vector.tensor_tensor(
                            out=wp[k][:], in0=wkl[:], in1=vpk[:], op=OP.mult
                        )

                if stage == "scans":
                    with tc.tile_pool(name="dmp", bufs=2) as dmp:
                        t = dmp.tile([128, 96], f32, name="dumps", tag="dumps")
                        for j, src in enumerate([sloti[0], sloti[1], gidxi[0], gidxi[1], wp[0], wp[1]]):
                            nc.vector.tensor_copy(t[:, j * 16 : (j + 1) * 16], src[:])
                        nc.sync.dma_start(out[0:128, 0:96], t[:])
                    return

                # ============ phase 5: scatter token rows into groups ============
                # The indirect DMA consumes exactly one offset per partition,
                # so this is one call per 128-token tile. Slots are unique per
                # assignment, so the calls write disjoint rows; a critical
                # section with manual completion semaphores lets them stream
                # back-to-back on the gpsimd queue instead of paying a
                # completion round-trip between each pair.
                if os.environ.get("MOE_SCRIT", "1") not in ("", "0"):
                    ssem = nc.alloc_semaphore("scat_sem")
                    with tc.tile_critical():
                        for k in range(2):
                            for tt in range(NTT):
                                nc.gpsimd.indirect_dma_start(
                                    out=xin,
                                    out_offset=IndirectOffsetOnAxis(
                                        ap=sloti[k][:, tt : tt + 1], axis=0
                                    ),
                                    in_=xsg3[:, tt, :],
                                    in_offset=None,
                                    bounds_check=nsl - 1,
                                    oob_is_err=False,
                                ).then_inc(ssem, 16)
                        nc.gpsimd.nop(nofuse=True, hint="scat_wait")._wait_ge(
                            ssem, 2 * NTT * 16
                        )
                else:
                    for k in range(2):
                        for tt in range(NTT):
                            nc.gpsimd.indirect_dma_start(
                                out=xin,
                                out_offset=IndirectOffsetOnAxis(
                                    ap=sloti[k][:, tt : tt + 1], axis=0
                                ),
                                in_=xsg3[:, tt, :],
                                in_offset=None,
                                bounds_check=nsl - 1,
                                oob_is_err=False,
                            )

            if stage == "scatter":
                with tc.tile_pool(name="dmp", bufs=2) as dmp:
                    _dump(tc, dmp, xin, H, dt=gdt)
                return

            # ================= phase 6: expert FFNs =================
            with (
                tc.tile_pool(name="fps_tp", bufs=2, space="PSUM") as fps_tp,
                tc.tile_pool(name="fps_gu", bufs=2, space="PSUM") as fps_gu,
                tc.tile_pool(name="fps_e", bufs=2, space="PSUM") as fps_e,
                tc.tile_pool(name="fsb", bufs=1) as fsb,
                tc.tile_pool(name="fw", bufs=2) as fw,
                tc.tile_pool(name="fio", bufs=2) as fio,
            ):
                cc = [(0, min(512, capl))]
                if capl > 512:
                    cc.append((512, capl - 512))

                def build_actT(e):
                    # build transposed activations on the PE (identity matmuls)
                    actT = fsb.tile([128, HT * capl], gdt, name="actT", tag="actT", bufs=3)
                    actT3 = actT[:].rearrange("p (h c) -> p h c", c=capl)
                    for r0c, rws in ctiles:
                        r0 = e * capl + r0c
                        xi = fio.tile([128, H], gdt, name="xin_sb", tag="xin_sb", bufs=3)
                        nc.sync.dma_start(xi[0:rws, :], xin[r0 : r0 + rws, :])
                        for h in range(HT):
                            tp = fps_tp.tile([128, 128], gdt, name="ffn_tp", tag="ffn_tp")
                            nc.tensor.transpose(
                                tp[:, 0:rws],
                                xi[0:rws, h * 128 : (h + 1) * 128],
                                ident_g[0:rws, 0:rws],
                            )
                            nc.vector.tensor_copy(
                                actT3[:, h, r0c : r0c + rws], tp[:, 0:rws]
                            )
                    return actT3

                actT3_cur = build_actT(0)
                for e in range(E):
                    actT3 = actT3_cur
                    hT = fsb.tile([128, FT * capl], gdt, name="hT", tag="hT", bufs=2)
                    hT3 = hT[:].rearrange("p (f c) -> p f c", c=capl)
                    for fg0 in range(0, FT, FG):
                        fgi = fg0 // FG
                        w1g = fw.tile([128, HT * FG * 128], gdt, name="w1g", tag="w1g")
                        w3g = fw.tile([128, HT * FG * 128], gdt, name="w3g", tag="w3g")
                        w1g3 = w1g[:].rearrange("p (h f) -> p h f", f=FG * 128)
                        w3g3 = w3g[:].rearrange("p (h f) -> p h f", f=FG * 128)
                        nc.sync.dma_start(w1g[:], w1[e, fgi])
                        nc.sync.dma_start(w3g[:], w3[e, fgi])
                        for ft in range(fg0, fg0 + FG):
                            fo = (ft - fg0) * 128
                            ga = [
                                fps_gu.tile([128, w_], f32, name=f"gu{ci}", tag=f"gu{ci}")
                                for ci, (_, w_) in enumerate(cc)
                            ]
                            for h in range(HT):
                                for ci, (c0, w_) in enumerate(cc):
                                    nc.tensor.matmul(
                                        ga[ci][:],
                                        lhsT=w1g3[:, h, fo : fo + 128],
                                        rhs=actT3[:, h, c0 : c0 + w_],
                                        start=(h == 0),
                                        stop=(h == HT - 1),
                                    )
                            # t = silu(g) = g * sigmoid(g)
                            tsl = fio.tile([128, capl], f32, name="tsilu", tag="tsilu")
                            for ci, (c0, w_) in enumerate(cc):
                                nc.scalar.activation(
                                    tsl[:, c0 : c0 + w_], ga[ci][:], AF.Sigmoid
                                )
                                nc.vector.tensor_tensor(
                                    out=tsl[:, c0 : c0 + w_],
                                    in0=tsl[:, c0 : c0 + w_],
                                    in1=ga[ci][:],
                                    op=OP.mult,
                                )
                            # u = x @ W3 (reuse psum slots)
                            ua = [
                                fps_gu.tile([128, w_], f32, name=f"gu{ci}", tag=f"gu{ci}")
                                for ci, (_, w_) in enumerate(cc)
                            ]
                            for h in range(HT):
                                for ci, (c0, w_) in enumerate(cc):
                                    nc.tensor.matmul(
                                        ua[ci][:],
                                        lhsT=w3g3[:, h, fo : fo + 128],
                                        rhs=actT3[:, h, c0 : c0 + w_],
                                        start=(h == 0),
                                        stop=(h == HT - 1),
                                    )
                            # hT = silu(g) * u
                            for ci, (c0, w_) in enumerate(cc):
                                nc.vector.tensor_tensor(
                                    out=hT3[:, ft, c0 : c0 + w_],
                                    in0=tsl[:, c0 : c0 + w_],
                                    in1=ua[ci][:],
                                    op=OP.mult,
                                )
                    # emit the next expert's activation-transpose build here so
                    # its DVE copies drain underneath pass 2's matmul stream
                    if e + 1 < E:
                        actT3_cur = build_actT(e + 1)
                    # pass 2: eout = hT.T @ W2
                    for hb in range(2):
                        w2r = fsb.tile([128, FT * 512], gdt, name="w2row", tag="w2row", bufs=2)
                        w2r3 = w2r[:].rearrange("p (f x) -> p f x", x=512)
                        nc.sync.dma_start(w2r[:], w2[e, hb])
                        eo = fio.tile([128, nct * 512], gdt, name="eo_sb", tag="eo_sb")
                        eo3 = eo[:].rearrange("p (n x) -> p n x", x=512)
                        for ci, (r0c, rws) in enumerate(ctiles):
                            eps = fps_e.tile([128, 512], f32, name="eps", tag="eps")
                            for ft in range(FT):
                                nc.tensor.matmul(
                                    eps[0:rws, :],
                                    lhsT=hT3[:, ft, r0c : r0c + rws],
                                    rhs=w2r3[:, ft, :],
                                    start=(ft == 0),
                                    stop=(ft == FT - 1),
                                )
                            nc.vector.tensor_copy(eo3[0:rws, ci, :], eps[0:rws, :])
                        # batched store per (e, hb): full tiles in one
                        # rearranged DMA, trailing partial tile separately
                        nc.sync.dma_start(
                            eout[e * capl : e * capl + nfull * 128,
                                 hb * 512 : (hb + 1) * 512]
                            .rearrange("(n p) x -> p n x", p=128),
                            eo3[:, 0:nfull, :],
                        )
                        if rem:
                            nc.sync.dma_start(
                                eout[e * capl + nfull * 128 : (e + 1) * capl,
                                     hb * 512 : (hb + 1) * 512],
                                eo3[0:rem, nfull, :],
                            )

                if stage == "ffn":
                    with tc.tile_pool(name="dmp2", bufs=2) as dmp2:
                        _dump(tc, dmp2, eout, H, dt=gdt)
                    return

                # ================= phase 7: combine =================
                for tt in range(NTT):
                    r1 = fio.tile([128, H], gdt, name="r1", tag="r1")
                    nc.gpsimd.indirect_dma_start(
                        out=r1[:],
                        out_offset=None,
                        in_=eout,
                        in_offset=IndirectOffsetOnAxis(
                            ap=gidxi[0][:, tt : tt + 1], axis=0
                        ),
                    )
                    r2 = fio.tile([128, H], gdt, name="r2", tag="r2")
                    nc.gpsimd.indirect_dma_start(
                        out=r2[:],
                        out_offset=None,
                        in_=eout,
                        in_offset=IndirectOffsetOnAxis(
                            ap=gidxi[1][:, tt : tt + 1], axis=0
                        ),
                    )
                    ot = fio.tile([128, H], f32, name="ot", tag="ot")
                    nc.vector.tensor_scalar(
                        out=ot[:],
                        in0=r1[:],
                        scalar1=wp[0][:, tt : tt + 1],
                        scalar2=None,
                        op0=OP.mult,
                    )
                    nc.vector.scalar_tensor_tensor(
                        out=ot[:],
                        in0=r2[:],
                        scalar=wp[1][:, tt : tt + 1],
                        in1=ot[:],
                        op0=OP.mult,
                        op1=OP.add,
                    )
                    nc.sync.dma_start(out[tt * 128 : (tt + 1) * 128, :], ot[:])

    return nc


_LDW_PATCHED = False


def _enable_ldw_opt():
    """Swap the hardcoded --enable-ldw-opt=false walrus flag to true: every
    fp32 matmul otherwise pays an unoverlapped LDWEIGHTS (~40% PE time)."""
    global _LDW_PATCHED
    if _LDW_PATCHED:
        return
    from concourse import bass_utils as _bu

    _orig = _bu.run_command

    def _patched(argv, **kw):
        argv = [
            a.replace("--enable-ldw-opt=false", "--enable-ldw-opt=true")
            if isinstance(a, str)
            else a
            for a in argv
        ]
        return _orig(argv, **kw)

    _bu.run_command = _patched
    _LDW_PATCHED = True


_NC_CACHE = {}


def _get_nc(capl, stage="full", use_bf16=True):
    # NOTE: --enable-ldw-opt=true crashes walrus codegen (visitInstLdweights,
    # CoreV3GenImpl.cpp:694) on this kernel — keep it off.
    if os.environ.get("MOE_LDW_OPT", "0") not in ("", "0"):
        _enable_ldw_opt()
    key = (capl, stage, use_bf16)
    if key not in _NC_CACHE:
        nc = bacc.Bacc("TRN2", debug=False, num_devices=NCORES)
        _emit(nc, capl, stage, use_bf16)
        nc.compile()
        _NC_CACHE[key] = nc
    return _NC_CACHE[key]


def _host_max_local_count(x, Wg):
    """Cheap host routing replica: max kept-assignments per (core, expert)."""
    logits = x.astype(np.float32) @ Wg.astype(np.float32)
    i1 = np.argmax(logits, axis=1)
    m = logits.copy()
    m[np.arange(T), i1] = -np.inf
    i2 = np.argmax(m, axis=1)
    routed = np.zeros((T, E), dtype=np.int64)
    routed[np.arange(T), i1] = 1
    routed[np.arange(T), i2] += 1
    pos = np.cumsum(routed, axis=0) - routed
    keep = routed * (pos < CAP)
    counts = keep.reshape(NCORES, TS, E).sum(axis=1)
    return int(counts.max())


def _install_ntff_hook():
    """Best-effort registration of the axon NTFF profiling hook (for tracing)."""
    import sys
    import types

    if "antenv.axon_hooks" in sys.modules:
        return
    try:
        mod = types.ModuleType("antenv.axon_hooks")
        hook = [None]
        mod.set_axon_ntff_profile_hook = lambda h: hook.__setitem__(0, h)
        mod.get_axon_ntff_profile_hook = lambda: hook[0]
        from trn_agent_boot.trn_boot import _ntff_profile_via_ctypes

        mod.set_axon_ntff_profile_hook(
            _ntff_profile_via_ctypes("/opt/axon/libaxon_pjrt.so")
        )
        sys.modules["antenv.axon_hooks"] = mod
    except Exception:
        pass


def kernel(x, Wg, W1, W3, W2):
    global LAST_RESULTS
    x = np.ascontiguousarray(np.asarray(x, dtype=np.float32))
    Wg = np.ascontiguousarray(np.asarray(Wg, dtype=np.float32))
    W1 = np.asarray(W1, dtype=np.float32)
    W3 = np.asarray(W3, dtype=np.float32)
    W2 = np.asarray(W2, dtype=np.float32)

    # static per-(core, expert) group capacity with safety margin (device
    # routing could diverge from this host estimate only on exact ties)
    maxc = _host_max_local_count(x, Wg)
    capl = max(128, int(math.ceil((maxc + 16) / 64.0) * 64))

    use_bf16 = os.environ.get("MOE_GDT", "bf16") != "f32"
    nc = _get_nc(capl, os.environ.get("MOE_STAGE", "full"), use_bf16)
    cst = _build_consts()
    wdt = np.float32
    if use_bf16:
        import ml_dtypes

        wdt = ml_dtypes.bfloat16
    # host-prepack the weight panels so each on-device load is one
    # contiguous-per-partition DMA:
    #   w1p[e, fg, p, (ht, fo)] = W1[e, ht*128+p, fg*256+fo]
    #   w2p[e, hb, p, (ft, x)]  = W2[e, ft*128+p, hb*512+x]
    w1p = np.ascontiguousarray(
        W1.reshape(E, HT, 128, NFG, FG * 128).transpose(0, 3, 2, 1, 4)
        .reshape(E, NFG, 128, HT * FG * 128).astype(wdt)
    )
    w3p = np.ascontiguousarray(
        W3.reshape(E, HT, 128, NFG, FG * 128).transpose(0, 3, 2, 1, 4)
        .reshape(E, NFG, 128, HT * FG * 128).astype(wdt)
    )
    w2p = np.ascontiguousarray(
        W2.reshape(E, FT, 128, 2, 512).transpose(0, 3, 2, 1, 4)
        .reshape(E, 2, 128, FT * 512).astype(wdt)
    )
    in_maps = []
    for c in range(NCORES):
        # premask indexing matches jm's (half, core, tile) column order:
        # entry p belongs to core (p % 64) // 8
        pm = ((np.arange(128) % 64) // 8 < c).astype(np.float32)[:, None]
        in_maps.append(
            {
                "xs": x[c * TS : (c + 1) * TS],
                "wg": Wg,
                "w1": w1p,
                "w3": w3p,
                "w2": w2p,
                "cst": cst,
                "premask": np.ascontiguousarray(pm),
            }
        )

    trace = os.environ.get("BASS_TRACE", "") not in ("", "0", "false", "False")
    if trace:
        _install_ntff_hook()
    res = run_bass_kernel_spmd(nc, in_maps, list(range(NCORES)), trace=trace)
    LAST_RESULTS = res
    return np.concatenate([res.results[c]["out"] for c in range(NCORES)], axis=0)


# revision 38
# speedup vs baseline: 1.0025x; 1.0010x over previous
"""MoE FFN layer (top-2 routing, SwiGLU experts) on 8 Trainium2 NeuronCores.

Sharding: data-parallel over tokens. Each core owns T/8 = 2048 tokens and a
replica of all expert weights. Routing is computed on-device:
  - each core computes gate logits + top-2 + softmax weights for its tokens
  - tiny AllGather of per-token routing info (idx1, idx2, w1, w2) [T, 4]
  - global per-expert capacity positions via matmul-based prefix scans
    (strict-upper-triangular-ones matmuls implement exclusive cumsum)
  - token rows are scattered into per-(core,expert) contiguous groups with one
    indirect DMA per top-k slot (out-of-bounds slot index = skip, which drops
    over-capacity assignments exactly like the reference)
  - per-expert SwiGLU GEMMs over the grouped rows; activations are transposed
    on the PE (identity matmuls) so no DMA-transpose xbar-mode serialization
  - weights are host-prepacked so each W1/W3 f-group and each W2 half-row
    panel is a single contiguous DMA
  - combine: indirect gather of each token's two expert-output rows (bf16) +
    weighted add; output shard is written densely, host concatenates shards.

The per-(core,expert) group capacity CAPL is chosen at call time from a cheap
host-side routing precheck (shapes must be static); the device still computes
all routing itself.  For the reference distribution counts are ~560, CAPL=640.
"""

import math
import os

import numpy as np

import concourse.bass as bass
import concourse.mybir as mybir
from concourse import bacc, tile
from concourse.bass import IndirectOffsetOnAxis
from concourse.bass_utils import run_bass_kernel_spmd

f32 = mybir.dt.float32
bf16 = mybir.dt.bfloat16
i32 = mybir.dt.int32
u32 = mybir.dt.uint32
AF = mybir.ActivationFunctionType
OP = mybir.AluOpType

T, H, F, E = 16384, 1024, 2816, 8
CAP = 5120  # global per-expert capacity = ceil(T * 1.25 * 2 / E)
NCORES = 8
TS = T // NCORES  # tokens per core (2048)
NTT = TS // 128  # token tiles per core (16)
NGT = T // 128  # global token tiles (128)
HT = H // 128  # 8
FT = F // 128  # 22
FG = 2  # f-tiles per W1/W3 weight DMA group
NFG = FT // FG  # 11
BIG = 1.0e6  # "invalid" slot marker, way past any bounds check

LAST_RESULTS = None  # BassKernelResults of the most recent run (for test.py)


CSTW = 520


def _build_consts():
    c = np.zeros((128, CSTW), dtype=np.float32)
    c[:, 0:128] = np.eye(128, dtype=np.float32)  # identity
    iu, ju = np.meshgrid(np.arange(128), np.arange(128), indexing="ij")
    c[:, 128:256] = (iu < ju).astype(np.float32)  # strict upper ones
    c[:, 256:384] = 1.0  # ones
    c[:, 384:392] = np.arange(8, dtype=np.float32)[None, :]  # iota8
    # block-strict: same expert (col%8), strictly earlier token tile (col//8)
    c[:, 392:520] = ((iu % 8 == ju % 8) & (iu // 8 < ju // 8)).astype(np.float32)
    return c


def _scan_all(nc, bps, sb_pool, in_view, ident, ustrict, bstrict, onescol, onesrow):
    """Exclusive prefix-sum over all 8 experts at once. ``in_view`` is
    [128 part, 128 cols] with col = n*8 + e; the scan for each expert e runs
    over its 16 n-columns in (n, partition) order. Returns a PSUM AP
    [128, 128] of per-element exclusive prefix sums."""
    pos = bps.tile([128, 128], f32, name="scan_pos", tag="scan_pos")
    # within-column strict prefix over partitions (all 128 cols at once)
    nc.tensor.matmul(pos[:], lhsT=ustrict, rhs=in_view, start=True, stop=False)
    # per-column sums -> [128 cols, 1]
    csT = bps.tile([128, 1], f32, name="scan_a", tag="scan_a")
    nc.tensor.matmul(csT[:], lhsT=in_view, rhs=onescol, start=True, stop=True)
    csT_sb = sb_pool.tile([128, 1], f32, name="scan_a_sb", tag="scan_a_sb")
    nc.vector.tensor_copy(csT_sb[:], csT[:])
    # exclusive prefix of column sums within each expert's column group
    excl = bps.tile([128, 1], f32, name="scan_b", tag="scan_b")
    nc.tensor.matmul(excl[:], lhsT=bstrict, rhs=csT_sb[:], start=True, stop=True)
    excl_sb = sb_pool.tile([128, 1], f32, name="scan_b_sb", tag="scan_b_sb")
    nc.vector.tensor_copy(excl_sb[:], excl[:])
    # transpose [128,1] -> [1,128]
    exclr = bps.tile([1, 128], f32, name="scan_c", tag="scan_c")
    nc.tensor.matmul(exclr[:], lhsT=excl_sb[:], rhs=ident, start=True, stop=True)
    exclr_sb = sb_pool.tile([1, 128], f32, name="scan_c_sb", tag="scan_c_sb")
    nc.vector.tensor_copy(exclr_sb[:], exclr[:])
    # broadcast the column offsets down all partitions
    nc.tensor.matmul(pos[:], lhsT=onesrow, rhs=exclr_sb[:], start=False, stop=True)
    return pos


def _emit(nc, capl, stage="full", use_bf16=True):
    nsl = E * capl
    nfull = capl // 128  # full 128-row c-tiles per expert group
    rem = capl % 128  # trailing partial c-tile rows (0 or 64)
    ctiles = [(i * 128, 128) for i in range(nfull)]
    if rem:
        ctiles.append((nfull * 128, rem))
    nct = len(ctiles)
    gdt = bf16 if use_bf16 else f32

    def _dump(tc, pool, dram_src, width, dt=f32):
        # copy dram_src[0:TS, 0:width] -> out rows via SBUF
        for i in range(NTT):
            t = pool.tile([128, width], dt, name="dumpt", tag="dumpt")
            nc.sync.dma_start(t[:], dram_src[i * 128 : (i + 1) * 128, 0:width])
            to = pool.tile([128, width], f32, name="dumpto", tag="dumpto")
            nc.vector.tensor_copy(to[:], t[:])
            nc.sync.dma_start(out[i * 128 : (i + 1) * 128, 0:width], to[:])

    xs = nc.dram_tensor("xs", [TS, H], f32, kind="ExternalInput").ap()
    wg = nc.dram_tensor("wg", [H, E], f32, kind="ExternalInput").ap()
    # host-prepacked weights: one contiguous DMA per (e, fg) / (e, hb) panel
    w1 = nc.dram_tensor("w1", [E, NFG, 128, HT * FG * 128], gdt, kind="ExternalInput").ap()
    w3 = nc.dram_tensor("w3", [E, NFG, 128, HT * FG * 128], gdt, kind="ExternalInput").ap()
    w2 = nc.dram_tensor("w2", [E, 2, 128, FT * 512], gdt, kind="ExternalInput").ap()
    cst = nc.dram_tensor("cst", [128, CSTW], f32, kind="ExternalInput").ap()
    premask = nc.dram_tensor("premask", [128, 1], f32, kind="ExternalInput").ap()
    out = nc.dram_tensor("out", [TS, H], f32, kind="ExternalOutput").ap()

    rloc = nc.dram_tensor("rloc", [TS, 2], f32).ap()
    # two-half AllGather outputs: half h holds all cores' token tiles
    # [h*8, h*8+8) in (core, tile, token) order
    rallA = nc.dram_tensor("rallA", [T // 2, 2], f32, addr_space="Shared").ap()
    rallB = nc.dram_tensor("rallB", [T // 2, 2], f32, addr_space="Shared").ap()
    xin = nc.dram_tensor("xin", [nsl, H], gdt).ap()
    eout = nc.dram_tensor("eout", [nsl, H], gdt).ap()

    with tile.TileContext(nc, num_cores=NCORES) as tc:
        with (
            tc.tile_pool(name="persist", bufs=1) as pp,
            tc.tile_pool(name="small", bufs=2) as sp,
        ):
            # ---- constants / static loads ----
            cst_sb = pp.tile([128, CSTW], f32, name="cst", tag="cst")
            nc.sync.dma_start(cst_sb[:], cst)
            ident = cst_sb[:, 0:128]
            ustrict = cst_sb[:, 128:256]
            onescol = cst_sb[:, 256:257]
            onesrow = cst_sb[0:1, 256:384]
            iota8 = cst_sb[:, 384:392]
            bstrict = cst_sb[:, 392:520]

            wg_sb = pp.tile([128, HT * E], f32, name="wg", tag="wg")
            nc.sync.dma_start(
                wg_sb[:].rearrange("p (n e) -> p n e", e=E),
                wg.rearrange("(n p) e -> p n e", p=128),
            )
            pm_sb = pp.tile([128, 1], f32, name="premask", tag="premask")
            nc.sync.dma_start(pm_sb[:], premask)

            # zero only the dummy landing block: slot 0 is the target of all
            # dropped/invalid gather indices, so row 0 must be finite.
            zsb = pp.tile([128, H], gdt, name="zeros", tag="zeros")
            nc.vector.memset(zsb[:], 0.0)
            nc.sync.dma_start(xin[0:128, :], zsb[:])

            ident_g = ident
            if use_bf16:
                identg_sb = pp.tile([128, 128], gdt, name="identg", tag="identg")
                nc.vector.tensor_copy(identg_sb[:], ident)
                ident_g = identg_sb[:]

            # persistent bookkeeping tiles
            jloc = pp.tile([128, NTT * E], f32, name="jloc", tag="jloc")
            oh1 = pp.tile([128, NTT * E], f32, name="oh1", tag="oh1")
            oh2 = pp.tile([128, NTT * E], f32, name="oh2", tag="oh2")
            i1f = pp.tile([128, NTT], f32, name="i1f", tag="i1f")
            i2f = pp.tile([128, NTT], f32, name="i2f", tag="i2f")
            w1l = pp.tile([128, NTT], f32, name="w1l", tag="w1l")
            w2l = pp.tile([128, NTT], f32, name="w2l", tag="w2l")
            vall = pp.tile([128, E * NTT], f32, name="vall", tag="vall")
            lrall = pp.tile([128, E * NTT], f32, name="lrall", tag="lrall")
            offb = pp.tile([128, E], f32, name="offb", tag="offb")
            idxb = pp.tile([128, NTT * 2], f32, name="idxb", tag="idxb")
            idxb3 = idxb[:].rearrange("p (n f) -> p n f", f=2)
            vbb = pp.tile([128, NTT * 2], f32, name="vbb", tag="vbb")
            vbb3 = vbb[:].rearrange("p (n f) -> p n f", f=2)
            sloti = [pp.tile([128, NTT], i32, name=f"slot{k}", tag=f"slot{k}") for k in range(2)]
            gidxi = [pp.tile([128, NTT], i32, name=f"gidx{k}", tag=f"gidx{k}") for k in range(2)]
            wp = [pp.tile([128, NTT], f32, name=f"wp{k}", tag=f"wp{k}") for k in range(2)]

            with tc.tile_pool(name="xspool", bufs=1) as xsp:
                xs_sb = xsp.tile([128, NTT * H], f32, name="xs", tag="xs")
                xs3 = xs_sb[:].rearrange("p (n h) -> p n h", h=H)
                if use_bf16:
                    xsg_sb = xsp.tile([128, NTT * H], gdt, name="xsg", tag="xsg")
                    xsg3 = xsg_sb[:].rearrange("p (n h) -> p n h", h=H)
                else:
                    xsg3 = xs3

                # ================= phase 1: gating =================
                with (
                    tc.tile_pool(name="gps", bufs=4, space="PSUM") as gps,
                    tc.tile_pool(name="gsb", bufs=4) as gsb,
                ):
                    for tt in range(NTT):
                        nc.sync.dma_start(
                            xs3[:, tt, :], xs[tt * 128 : (tt + 1) * 128, :]
                        )
                        if use_bf16:
                            # cast on the otherwise-idle scalar engine
                            nc.scalar.activation(
                                xsg3[:, tt, :], xs3[:, tt, :], AF.Copy
                            )
                        lg = gps.tile([128, E], f32, name="logits", tag="logits")
                        for hg in range(0, HT, 4):
                            # 4 transposes share one PSUM bank tile -> one
                            # wide copy, coarse PE<->DVE ping-pong
                            tpg = gps.tile([128, 512], f32, name="tpg", tag="tpg")
                            for j in range(4):
                                h = hg + j
                                nc.tensor.transpose(
                                    tpg[:, j * 128 : (j + 1) * 128],
                                    xs3[:, tt, h * 128 : (h + 1) * 128],
                                    ident,
                                )
                            xtg = gsb.tile([128, 512], f32, name="xtg", tag="xtg")
                            nc.vector.tensor_copy(xtg[:], tpg[:])
                            for j in range(4):
                                h = hg + j
                                nc.tensor.matmul(
                                    lg[:],
                                    lhsT=xtg[:, j * 128 : (j + 1) * 128],
                                    rhs=wg_sb[:].rearrange("p (n e) -> p n e", e=E)[
                                        :, h, :
                                    ],
                                    start=(h == 0),
                                    stop=(h == HT - 1),
                                )
                        lgs = gsb.tile([128, E], f32, name="lgs", tag="lgs")
                        nc.vector.tensor_copy(lgs[:], lg[:])
                        v8 = gsb.tile([128, 8], f32, name="v8", tag="v8")
                        nc.vector.max(out=v8[:], in_=lgs[:])
                        i8 = gsb.tile([128, 8], u32, name="i8", tag="i8")
                        nc.vector.max_index(out=i8[:], in_max=v8[:], in_values=lgs[:])
                        nc.vector.tensor_copy(idxb3[:, tt, :], i8[:, 0:2])
                        nc.vector.tensor_copy(vbb3[:, tt, :], v8[:, 0:2])
                        nc.sync.dma_start(
                            rloc[tt * 128 : (tt + 1) * 128, :], idxb3[:, tt, :]
                        )
                        if tt == NTT // 2 - 1:
                            # first-half routing is final: allgather it while
                            # the second half of gating runs
                            nc.gpsimd.collective_compute(
                                "AllGather",
                                OP.bypass,
                                replica_groups=[list(range(NCORES))],
                                ins=[rloc[0 : TS // 2, :]],
                                outs=[rallA],
                            )
                    # batched top-2 softmax over all tiles: w1 = 1/(1+d),
                    # w2 = d/(1+d) with d = exp(v2 - v1)
                    dd = gsb.tile([128, NTT], f32, name="dd", tag="dd")
                    nc.vector.tensor_tensor(
                        out=dd[:], in0=vbb3[:, :, 1], in1=vbb3[:, :, 0],
                        op=OP.subtract,
                    )
                    nc.scalar.activation(dd[:], dd[:], AF.Exp)
                    dp1 = gsb.tile([128, NTT], f32, name="dp1", tag="dp1")
                    nc.vector.tensor_scalar_add(dp1[:], dd[:], 1.0)
                    nc.vector.reciprocal(w1l[:], dp1[:])
                    nc.vector.tensor_tensor(
                        out=w2l[:], in0=dd[:], in1=w1l[:], op=OP.mult
                    )

                if stage == "gating":
                    with tc.tile_pool(name="dmp", bufs=2) as dmp:
                        _dump(tc, dmp, rloc, 2)
                    return

                # ============ phase 2+3: allgather + routing matrices ========
                with (
                    tc.tile_pool(name="bps", bufs=1, space="PSUM") as bps,
                    tc.tile_pool(name="ssb", bufs=3) as ssb,
                ):
                    # local routing blocks -> jloc/oh1/oh2/i tiles and the
                    # local scan; all of it reads only this core's routing, so
                    # it is emitted before the collective and overlaps it
                    jloc3 = jloc[:].rearrange("p (n e) -> p n e", e=E)
                    oh13 = oh1[:].rearrange("p (n e) -> p n e", e=E)
                    oh23 = oh2[:].rearrange("p (n e) -> p n e", e=E)
                    nc.vector.tensor_tensor(
                        out=oh13,
                        in0=idxb3[:, :, 0:1].broadcast_to([128, NTT, 8]),
                        in1=iota8.unsqueeze(1).broadcast_to([128, NTT, 8]),
                        op=OP.is_equal,
                    )
                    nc.vector.tensor_tensor(
                        out=oh23,
                        in0=idxb3[:, :, 1:2].broadcast_to([128, NTT, 8]),
                        in1=iota8.unsqueeze(1).broadcast_to([128, NTT, 8]),
                        op=OP.is_equal,
                    )
                    nc.vector.tensor_copy(i1f[:], idxb3[:, :, 0])
                    nc.vector.tensor_copy(i2f[:], idxb3[:, :, 1])
                    nc.vector.tensor_tensor(
                        out=jloc[:], in0=oh1[:], in1=oh2[:], op=OP.add
                    )
                    # batched local scan over all experts (position within
                    # this core's tokens, (n, e) column layout)
                    pos = _scan_all(
                        nc, bps, ssb, jloc[:], ident, ustrict, bstrict,
                        onescol, onesrow,
                    )
                    pos_sb = ssb.tile([128, NTT * E], f32, name="pos_sb", tag="pos_sb")
                    nc.vector.tensor_copy(pos_sb[:], pos[:])

                    nc.gpsimd.collective_compute(
                        "AllGather",
                        OP.bypass,
                        replica_groups=[list(range(NCORES))],
                        ins=[rloc[TS // 2 : TS, :]],
                        outs=[rallB],
                    )

                    # global per-(tile, expert) membership matrix [128, 128*8].
                    # Column groups are ordered (half, core, tile-within-half)
                    # so each AllGather half lands contiguously; the premask
                    # (earlier-core weighting) uses the same ordering.
                    jm = pp.tile([128, NGT * E], f32, name="jm", tag="jm")
                    jm3 = jm[:].rearrange("p (n e) -> p n e", e=E)
                    rsb = ssb.tile([128, NGT * 2], f32, name="rsb", tag="rsb")
                    rsb3 = rsb[:].rearrange("p (n f) -> p n f", f=2)
                    nc.sync.dma_start(
                        rsb3[:, 0 : NGT // 2, :],
                        rallA.rearrange("(g p) f -> p g f", p=128),
                    )
                    nc.sync.dma_start(
                        rsb3[:, NGT // 2 : NGT, :],
                        rallB.rearrange("(g p) f -> p g f", p=128),
                    )
                    ohb = ssb.tile([128, NGT * E], f32, name="ohb", tag="ohb")
                    ohb3 = ohb[:].rearrange("p (n e) -> p n e", e=E)
                    nc.vector.tensor_tensor(
                        out=jm3,
                        in0=rsb3[:, :, 0:1].broadcast_to([128, NGT, 8]),
                        in1=iota8.unsqueeze(1).broadcast_to([128, NGT, 8]),
                        op=OP.is_equal,
                    )
                    nc.vector.tensor_tensor(
                        out=ohb3,
                        in0=rsb3[:, :, 1:2].broadcast_to([128, NGT, 8]),
                        in1=iota8.unsqueeze(1).broadcast_to([128, NGT, 8]),
                        op=OP.is_equal,
                    )
                    nc.vector.tensor_tensor(
                        out=jm[:], in0=jm[:], in1=ohb[:], op=OP.add
                    )

                    # per-expert base offsets: count of earlier-core tokens
                    offr = bps.tile([1, E], f32, name="offrow", tag="offrow")
                    for e in range(E):
                        csall = bps.tile([128, 1], f32, name="csall", tag="csall", bufs=2)
                        nc.tensor.matmul(
                            csall[:],
                            lhsT=jm3[:, :, e],
                            rhs=onescol,
                            start=True,
                            stop=True,
                        )
                        csall_sb = ssb.tile([128, 1], f32, name="csall_sb", tag="csall_sb")
                        nc.vector.tensor_copy(csall_sb[:], csall[:])
                        nc.tensor.matmul(
                            offr[0:1, e : e + 1],
                            lhsT=csall_sb[:],
                            rhs=pm_sb[:],
                            start=True,
                            stop=True,
                        )
                    offr_sb = ssb.tile([1, E], f32, name="offr_sb", tag="offr_sb")
                    nc.vector.tensor_copy(offr_sb[:], offr[:])
                    offbp = bps.tile([128, E], f32, name="offbp", tag="offbp")
                    nc.tensor.matmul(
                        offbp[:], lhsT=onesrow, rhs=offr_sb[:], start=True, stop=True
                    )
                    nc.vector.tensor_copy(offb[:], offbp[:])

                    # global position -> validity -> local rank
                    gpos = ssb.tile([128, NTT * E], f32, name="gpos", tag="gpos")
                    nc.vector.tensor_tensor(
                        out=gpos[:].rearrange("p (n e) -> p n e", e=E),
                        in0=pos_sb[:].rearrange("p (n e) -> p n e", e=E),
                        in1=offb[:].unsqueeze(1).broadcast_to([128, NTT, E]),
                        op=OP.add,
                    )
                    nc.vector.scalar_tensor_tensor(
                        out=vall[:],
                        in0=gpos[:],
                        scalar=float(CAP),
                        in1=jloc[:],
                        op0=OP.is_lt,
                        op1=OP.mult,
                    )
                    lr = _scan_all(
                        nc, bps, ssb, vall[:], ident, ustrict, bstrict,
                        onescol, onesrow,
                    )
                    nc.vector.tensor_copy(lrall[:], lr[:])

                    # ---- per-assignment slot / gather-index / weight ----
                    for k, (ikf, ohk, wkl) in enumerate(
                        [(i1f, oh1, w1l), (i2f, oh2, w2l)]
                    ):
                        lrp = ssb.tile([128, NTT], f32, name=f"lrp{k}", tag=f"lrp{k}")
                        vpk = ssb.tile([128, NTT], f32, name=f"vpk{k}", tag=f"vpk{k}")
                        tmp = ssb.tile([128, NTT], f32, name=f"tmp{k}", tag=f"tmp{k}")
                        t128 = ssb.tile([128, NTT * E], f32, name=f"t128_{k}", tag=f"t128_{k}")
                        nc.vector.tensor_tensor(
                            out=t128[:], in0=ohk[:], in1=lrall[:], op=OP.mult
                        )
                        nc.vector.tensor_reduce(
                            out=lrp[:],
                            in_=t128[:].rearrange("p (n e) -> p n e", e=E),
                            axis=mybir.AxisListType.X,
                            op=OP.add,
                        )
                        nc.vector.tensor_tensor(
                            out=t128[:], in0=ohk[:], in1=vall[:], op=OP.mult
                        )
                        nc.vector.tensor_reduce(
                            out=vpk[:],
                            in_=t128[:].rearrange("p (n e) -> p n e", e=E),
                            axis=mybir.AxisListType.X,
                            op=OP.add,
                        )
                        # slot = e*capl + lrank, or >= nsl when invalid
                        slot = ssb.tile([128, NTT], f32, name=f"slotf{k}", tag=f"slotf{k}")
                        nc.vector.scalar_tensor_tensor(
                            out=slot[:],
                            in0=ikf[:],
                            scalar=float(capl),
                            in1=lrp[:],
                            op0=OP.mult,
                            op1=OP.add,
                        )
                        nc.vector.tensor_scalar(
                            out=tmp[:],
                            in0=vpk[:],
                            scalar1=0.5,
                            scalar2=None,
                            op0=OP.is_lt,
                        )
                        nc.vector.scalar_tensor_tensor(
                            out=slot[:],
                            in0=tmp[:],
                            scalar=BIG,
                            in1=slot[:],
                            op0=OP.mult,
                            op1=OP.add,
                        )
                        nc.vector.tensor_copy(sloti[k][:], slot[:])
                        # gather idx = clamped slot, 0 when invalid
                        nc.vector.tensor_scalar_min(tmp[:], lrp[:], float(capl - 1))
                        nc.vector.scalar_tensor_tensor(
                            out=tmp[:],
                            in0=ikf[:],
                            scalar=float(capl),
                            in1=tmp[:],
                            op0=OP.mult,
                            op1=OP.add,
                        )
                        nc.vector.tensor_tensor(
                            out=tmp[:], in0=tmp[:], in1=vpk[:], op=OP.mult
                        )
                        nc.vector.tensor_copy(gidxi[k][:], tmp[:])
                        # combine weight = w_k * valid
                        nc.vector.tensor_tensor(
                            out=wp[k][:], in0=wkl[:], in1=vpk[:], op=OP.mult
                        )

                if stage == "scans":
                    with tc.tile_pool(name="dmp", bufs=2) as dmp:
                        t = dmp.tile([128, 96], f32, name="dumps", tag="dumps")
                        for j, src in enumerate([sloti[0], sloti[1], gidxi[0], gidxi[1], wp[0], wp[1]]):
                            nc.vector.tensor_copy(t[:, j * 16 : (j + 1) * 16], src[:])
                        nc.sync.dma_start(out[0:128, 0:96], t[:])
                    return

                # ============ phase 5: scatter token rows into groups ============
                # The indirect DMA consumes exactly one offset per partition,
                # so this is one call per 128-token tile. Slots are unique per
                # assignment, so the calls write disjoint rows; a critical
                # section with manual completion semaphores lets them stream
                # back-to-back on the gpsimd queue instead of paying a
                # completion round-trip between each pair.
                if os.environ.get("MOE_SCRIT", "1") not in ("", "0"):
                    ssem = nc.alloc_semaphore("scat_sem")
                    with tc.tile_critical():
                        for k in range(2):
                            for tt in range(NTT):
                                nc.gpsimd.indirect_dma_start(
                                    out=xin,
                                    out_offset=IndirectOffsetOnAxis(
                                        ap=sloti[k][:, tt : tt + 1], axis=0
                                    ),
                                    in_=xsg3[:, tt, :],
                                    in_offset=None,
                                    bounds_check=nsl - 1,
                                    oob_is_err=False,
                                ).then_inc(ssem, 16)
                        nc.gpsimd.nop(nofuse=True, hint="scat_wait")._wait_ge(
                            ssem, 2 * NTT * 16
                        )
                else:
                    for k in range(2):
                        for tt in range(NTT):
                            nc.gpsimd.indirect_dma_start(
                                out=xin,
                                out_offset=IndirectOffsetOnAxis(
                                    ap=sloti[k][:, tt : tt + 1], axis=0
                                ),
                                in_=xsg3[:, tt, :],
                                in_offset=None,
                                bounds_check=nsl - 1,
                                oob_is_err=False,
                            )

            if stage == "scatter":
                with tc.tile_pool(name="dmp", bufs=2) as dmp:
                    _dump(tc, dmp, xin, H, dt=gdt)
                return

            # ================= phase 6: expert FFNs =================
            with (
                tc.tile_pool(name="fps_tp", bufs=2, space="PSUM") as fps_tp,
                tc.tile_pool(name="fps_gu", bufs=2, space="PSUM") as fps_gu,
                tc.tile_pool(name="fps_e", bufs=2, space="PSUM") as fps_e,
                tc.tile_pool(name="fsb", bufs=1) as fsb,
                tc.tile_pool(name="fw", bufs=2) as fw,
                tc.tile_pool(name="fio", bufs=2) as fio,
            ):
                cc = [(0, min(512, capl))]
                if capl > 512:
                    cc.append((512, capl - 512))

                def build_actT(e):
                    # build transposed activations on the PE (identity matmuls)
                    actT = fsb.tile([128, HT * capl], gdt, name="actT", tag="actT", bufs=3)
                    actT3 = actT[:].rearrange("p (h c) -> p h c", c=capl)
                    for r0c, rws in ctiles:
                        r0 = e * capl + r0c
                        xi = fio.tile([128, H], gdt, name="xin_sb", tag="xin_sb", bufs=3)
                        nc.sync.dma_start(xi[0:rws, :], xin[r0 : r0 + rws, :])
                        for h in range(HT):
                            tp = fps_tp.tile([128, 128], gdt, name="ffn_tp", tag="ffn_tp")
                            nc.tensor.transpose(
                                tp[:, 0:rws],
                                xi[0:rws, h * 128 : (h + 1) * 128],
                                ident_g[0:rws, 0:rws],
                            )
                            nc.vector.tensor_copy(
                                actT3[:, h, r0c : r0c + rws], tp[:, 0:rws]
                            )
                    return actT3

                actT3_cur = build_actT(0)
                for e in range(E):
                    actT3 = actT3_cur
                    hT = fsb.tile([128, FT * capl], gdt, name="hT", tag="hT", bufs=2)
                    hT3 = hT[:].rearrange("p (f c) -> p f c", c=capl)
                    for fg0 in range(0, FT, FG):
                        fgi = fg0 // FG
                        w1g = fw.tile([128, HT * FG * 128], gdt, name="w1g", tag="w1g")
                        w3g = fw.tile([128, HT * FG * 128], gdt, name="w3g", tag="w3g")
                        w1g3 = w1g[:].rearrange("p (h f) -> p h f", f=FG * 128)
                        w3g3 = w3g[:].rearrange("p (h f) -> p h f", f=FG * 128)
                        nc.sync.dma_start(w1g[:], w1[e, fgi])
                        nc.sync.dma_start(w3g[:], w3[e, fgi])
                        for ft in range(fg0, fg0 + FG):
                            fo = (ft - fg0) * 128
                            ga = [
                                fps_gu.tile([128, w_], f32, name=f"gu{ci}", tag=f"gu{ci}")
                                for ci, (_, w_) in enumerate(cc)
                            ]
                            for h in range(HT):
                                for ci, (c0, w_) in enumerate(cc):
                                    nc.tensor.matmul(
                                        ga[ci][:],
                                        lhsT=w1g3[:, h, fo : fo + 128],
                                        rhs=actT3[:, h, c0 : c0 + w_],
                                        start=(h == 0),
                                        stop=(h == HT - 1),
                                    )
                            # t = silu(g) = g * sigmoid(g)
                            tsl = fio.tile([128, capl], f32, name="tsilu", tag="tsilu")
                            for ci, (c0, w_) in enumerate(cc):
                                nc.scalar.activation(
                                    tsl[:, c0 : c0 + w_], ga[ci][:], AF.Sigmoid
                                )
                                nc.vector.tensor_tensor(
                                    out=tsl[:, c0 : c0 + w_],
                                    in0=tsl[:, c0 : c0 + w_],
                                    in1=ga[ci][:],
                                    op=OP.mult,
                                )
                            # u = x @ W3 (reuse psum slots)
                            ua = [
                                fps_gu.tile([128, w_], f32, name=f"gu{ci}", tag=f"gu{ci}")
                                for ci, (_, w_) in enumerate(cc)
                            ]
                            for h in range(HT):
                                for ci, (c0, w_) in enumerate(cc):
                                    nc.tensor.matmul(
                                        ua[ci][:],
                                        lhsT=w3g3[:, h, fo : fo + 128],
                                        rhs=actT3[:, h, c0 : c0 + w_],
                                        start=(h == 0),
                                        stop=(h == HT - 1),
                                    )
                            # hT = silu(g) * u
                            for ci, (c0, w_) in enumerate(cc):
                                nc.vector.tensor_tensor(
                                    out=hT3[:, ft, c0 : c0 + w_],
                                    in0=tsl[:, c0 : c0 + w_],
                                    in1=ua[ci][:],
                                    op=OP.mult,
                                )
                    # emit the next expert's activation-transpose build here so
                    # its DVE copies drain underneath pass 2's matmul stream
                    if e + 1 < E:
                        actT3_cur = build_actT(e + 1)
                    # pass 2: eout = hT.T @ W2
                    for hb in range(2):
                        w2r = fsb.tile([128, FT * 512], gdt, name="w2row", tag="w2row", bufs=2)
                        w2r3 = w2r[:].rearrange("p (f x) -> p f x", x=512)
                        nc.sync.dma_start(w2r[:], w2[e, hb])
                        eo = fio.tile([128, nct * 512], gdt, name="eo_sb", tag="eo_sb")
                        eo3 = eo[:].rearrange("p (n x) -> p n x", x=512)
                        for ci, (r0c, rws) in enumerate(ctiles):
                            eps = fps_e.tile([128, 512], f32, name="eps", tag="eps")
                            for ft in range(FT):
                                nc.tensor.matmul(
                                    eps[0:rws, :],
                                    lhsT=hT3[:, ft, r0c : r0c + rws],
                                    rhs=w2r3[:, ft, :],
                                    start=(ft == 0),
                                    stop=(ft == FT - 1),
                                )
                            nc.vector.tensor_copy(eo3[0:rws, ci, :], eps[0:rws, :])
                        # batched store per (e, hb): full tiles in one
                        # rearranged DMA, trailing partial tile separately
                        nc.sync.dma_start(
                            eout[e * capl : e * capl + nfull * 128,
                                 hb * 512 : (hb + 1) * 512]
                            .rearrange("(n p) x -> p n x", p=128),
                            eo3[:, 0:nfull, :],
                        )
                        if rem:
                            nc.sync.dma_start(
                                eout[e * capl + nfull * 128 : (e + 1) * capl,
                                     hb * 512 : (hb + 1) * 512],
                                eo3[0:rem, nfull, :],
                            )

                if stage == "ffn":
                    with tc.tile_pool(name="dmp2", bufs=2) as dmp2:
                        _dump(tc, dmp2, eout, H, dt=gdt)
                    return

                # ================= phase 7: combine =================
                for tt in range(NTT):
                    r1 = fio.tile([128, H], gdt, name="r1", tag="r1")
                    nc.gpsimd.indirect_dma_start(
                        out=r1[:],
                        out_offset=None,
                        in_=eout,
                        in_offset=IndirectOffsetOnAxis(
                            ap=gidxi[0][:, tt : tt + 1], axis=0
                        ),
                    )
                    r2 = fio.tile([128, H], gdt, name="r2", tag="r2")
                    nc.gpsimd.indirect_dma_start(
                        out=r2[:],
                        out_offset=None,
                        in_=eout,
                        in_offset=IndirectOffsetOnAxis(
                            ap=gidxi[1][:, tt : tt + 1], axis=0
                        ),
                    )
                    ot = fio.tile([128, H], f32, name="ot", tag="ot")
                    nc.vector.tensor_scalar(
                        out=ot[:],
                        in0=r1[:],
                        scalar1=wp[0][:, tt : tt + 1],
                        scalar2=None,
                        op0=OP.mult,
                    )
                    nc.vector.scalar_tensor_tensor(
                        out=ot[:],
                        in0=r2[:],
                        scalar=wp[1][:, tt : tt + 1],
                        in1=ot[:],
                        op0=OP.mult,
                        op1=OP.add,
                    )
                    nc.sync.dma_start(out[tt * 128 : (tt + 1) * 128, :], ot[:])

    return nc


_LDW_PATCHED = False


def _enable_ldw_opt():
    """Swap the hardcoded --enable-ldw-opt=false walrus flag to true: every
    fp32 matmul otherwise pays an unoverlapped LDWEIGHTS (~40% PE time)."""
    global _LDW_PATCHED
    if _LDW_PATCHED:
        return
    from concourse import bass_utils as _bu

    _orig = _bu.run_command

    def _patched(argv, **kw):
        argv = [
            a.replace("--enable-ldw-opt=false", "--enable-ldw-opt=true")
            if isinstance(a, str)
            else a
            for a in argv
        ]
        return _orig(argv, **kw)

    _bu.run_command = _patched
    _LDW_PATCHED = True


_NC_CACHE = {}


def _get_nc(capl, stage="full", use_bf16=True):
    # NOTE: --enable-ldw-opt=true crashes walrus codegen (visitInstLdweights,
    # CoreV3GenImpl.cpp:694) on this kernel — keep it off.
    if os.environ.get("MOE_LDW_OPT", "0") not in ("", "0"):
        _enable_ldw_opt()
    key = (capl, stage, use_bf16)
    if key not in _NC_CACHE:
        nc = bacc.Bacc("TRN2", debug=False, num_devices=NCORES)
        _emit(nc, capl, stage, use_bf16)
        nc.compile()
        _NC_CACHE[key] = nc
    return _NC_CACHE[key]


def _host_max_local_count(x, Wg):
    """Cheap host routing replica: max kept-assignments per (core, expert)."""
    logits = x.astype(np.float32) @ Wg.astype(np.float32)
    i1 = np.argmax(logits, axis=1)
    m = logits.copy()
    m[np.arange(T), i1] = -np.inf
    i2 = np.argmax(m, axis=1)
    routed = np.zeros((T, E), dtype=np.int64)
    routed[np.arange(T), i1] = 1
    routed[np.arange(T), i2] += 1
    pos = np.cumsum(routed, axis=0) - routed
    keep = routed * (pos < CAP)
    counts = keep.reshape(NCORES, TS, E).sum(axis=1)
    return int(counts.max())


def _install_ntff_hook():
    """Best-effort registration of the axon NTFF profiling hook (for tracing)."""
    import sys
    import types

    if "antenv.axon_hooks" in sys.modules:
        return
    try:
        mod = types.ModuleType("antenv.axon_hooks")
        hook = [None]
        mod.set_axon_ntff_profile_hook = lambda h: hook.__setitem__(0, h)
        mod.get_axon_ntff_profile_hook = lambda: hook[0]
        from trn_agent_boot.trn_boot import _ntff_profile_via_ctypes

        mod.set_axon_ntff_profile_hook(
            _ntff_profile_via_ctypes("/opt/axon/libaxon_pjrt.so")
        )
        sys.modules["antenv.axon_hooks"] = mod
    except Exception:
        pass


def kernel(x, Wg, W1, W3, W2):
    global LAST_RESULTS
    x = np.ascontiguousarray(np.asarray(x, dtype=np.float32))
    Wg = np.ascontiguousarray(np.asarray(Wg, dtype=np.float32))
    W1 = np.asarray(W1, dtype=np.float32)
    W3 = np.asarray(W3, dtype=np.float32)
    W2 = np.asarray(W2, dtype=np.float32)

    # static per-(core, expert) group capacity with safety margin (device
    # routing could diverge from this host estimate only on exact ties)
    maxc = _host_max_local_count(x, Wg)
    capl = max(128, int(math.ceil((maxc + 16) / 64.0) * 64))

    use_bf16 = os.environ.get("MOE_GDT", "bf16") != "f32"
    nc = _get_nc(capl, os.environ.get("MOE_STAGE", "full"), use_bf16)
    cst = _build_consts()
    wdt = np.float32
    if use_bf16:
        import ml_dtypes

        wdt = ml_dtypes.bfloat16
    # host-prepack the weight panels so each on-device load is one
    # contiguous-per-partition DMA:
    #   w1p[e, fg, p, (ht, fo)] = W1[e, ht*128+p, fg*256+fo]
    #   w2p[e, hb, p, (ft, x)]  = W2[e, ft*128+p, hb*512+x]
    w1p = np.ascontiguousarray(
        W1.reshape(E, HT, 128, NFG, FG * 128).transpose(0, 3, 2, 1, 4)
        .reshape(E, NFG, 128, HT * FG * 128).astype(wdt)
    )
    w3p = np.ascontiguousarray(
        W3.reshape(E, HT, 128, NFG, FG * 128).transpose(0, 3, 2, 1, 4)
        .reshape(E, NFG, 128, HT * FG * 128).astype(wdt)
    )
    w2p = np.ascontiguousarray(
        W2.reshape(E, FT, 128, 2, 512).transpose(0, 3, 2, 1, 4)
        .reshape(E, 2, 128, FT * 512).astype(wdt)
    )
    in_maps = []
    for c in range(NCORES):
        # premask indexing matches jm's (half, core, tile) column order:
        # entry p belongs to core (p % 64) // 8
        pm = ((np.arange(128) % 64) // 8 < c).astype(np.float32)[:, None]
        in_maps.append(
            {
                "xs": x[c * TS : (c + 1) * TS],
                "wg": Wg,
                "w1": w1p,
                "w3": w3p,
                "w2": w2p,
                "cst": cst,
                "premask": np.ascontiguousarray(pm),
            }
        )

    trace = os.environ.get("BASS_TRACE", "") not in ("", "0", "false", "False")
    if trace:
        _install_ntff_hook()
    res = run_bass_kernel_spmd(nc, in_maps, list(range(NCORES)), trace=trace)
    LAST_RESULTS = res
    return np.concatenate([res.results[c]["out"] for c in range(NCORES)], axis=0)


# revision 40
# speedup vs baseline: 1.0087x; 1.0061x over previous
"""MoE FFN layer (top-2 routing, SwiGLU experts) on 8 Trainium2 NeuronCores.

Sharding: data-parallel over tokens. Each core owns T/8 = 2048 tokens and a
replica of all expert weights. Routing is computed on-device:
  - each core computes gate logits + top-2 + softmax weights for its tokens
  - tiny AllGather of per-token routing info (idx1, idx2, w1, w2) [T, 4]
  - global per-expert capacity positions via matmul-based prefix scans
    (strict-upper-triangular-ones matmuls implement exclusive cumsum)
  - token rows are scattered into per-(core,expert) contiguous groups with one
    indirect DMA per top-k slot (out-of-bounds slot index = skip, which drops
    over-capacity assignments exactly like the reference)
  - per-expert SwiGLU GEMMs over the grouped rows; activations are transposed
    on the PE (identity matmuls) so no DMA-transpose xbar-mode serialization
  - weights are host-prepacked so each W1/W3 f-group and each W2 half-row
    panel is a single contiguous DMA
  - combine: indirect gather of each token's two expert-output rows (bf16) +
    weighted add; output shard is written densely, host concatenates shards.

The per-(core,expert) group capacity CAPL is chosen at call time from a cheap
host-side routing precheck (shapes must be static); the device still computes
all routing itself.  For the reference distribution counts are ~560, CAPL=640.
"""

import math
import os

import numpy as np

import concourse.bass as bass
import concourse.mybir as mybir
from concourse import bacc, tile
from concourse.bass import IndirectOffsetOnAxis
from concourse.bass_utils import run_bass_kernel_spmd

f32 = mybir.dt.float32
bf16 = mybir.dt.bfloat16
i32 = mybir.dt.int32
u32 = mybir.dt.uint32
AF = mybir.ActivationFunctionType
OP = mybir.AluOpType

T, H, F, E = 16384, 1024, 2816, 8
CAP = 5120  # global per-expert capacity = ceil(T * 1.25 * 2 / E)
NCORES = 8
TS = T // NCORES  # tokens per core (2048)
NTT = TS // 128  # token tiles per core (16)
NGT = T // 128  # global token tiles (128)
HT = H // 128  # 8
FT = F // 128  # 22
FG = 2  # f-tiles per W1/W3 weight DMA group
NFG = FT // FG  # 11
BIG = 1.0e6  # "invalid" slot marker, way past any bounds check

LAST_RESULTS = None  # BassKernelResults of the most recent run (for test.py)


CSTW = 520


def _build_consts():
    c = np.zeros((128, CSTW), dtype=np.float32)
    c[:, 0:128] = np.eye(128, dtype=np.float32)  # identity
    iu, ju = np.meshgrid(np.arange(128), np.arange(128), indexing="ij")
    c[:, 128:256] = (iu < ju).astype(np.float32)  # strict upper ones
    c[:, 256:384] = 1.0  # ones
    c[:, 384:392] = np.arange(8, dtype=np.float32)[None, :]  # iota8
    # block-strict: same expert (col%8), strictly earlier token tile (col//8)
    c[:, 392:520] = ((iu % 8 == ju % 8) & (iu // 8 < ju // 8)).astype(np.float32)
    return c


def _scan_all(nc, bps, sb_pool, in_view, ident, ustrict, bstrict, onescol, onesrow):
    """Exclusive prefix-sum over all 8 experts at once. ``in_view`` is
    [128 part, 128 cols] with col = n*8 + e; the scan for each expert e runs
    over its 16 n-columns in (n, partition) order. Returns a PSUM AP
    [128, 128] of per-element exclusive prefix sums."""
    pos = bps.tile([128, 128], f32, name="scan_pos", tag="scan_pos")
    # within-column strict prefix over partitions (all 128 cols at once)
    nc.tensor.matmul(pos[:], lhsT=ustrict, rhs=in_view, start=True, stop=False)
    # per-column sums -> [128 cols, 1]
    csT = bps.tile([128, 1], f32, name="scan_a", tag="scan_a")
    nc.tensor.matmul(csT[:], lhsT=in_view, rhs=onescol, start=True, stop=True)
    csT_sb = sb_pool.tile([128, 1], f32, name="scan_a_sb", tag="scan_a_sb")
    nc.vector.tensor_copy(csT_sb[:], csT[:])
    # exclusive prefix of column sums within each expert's column group
    excl = bps.tile([128, 1], f32, name="scan_b", tag="scan_b")
    nc.tensor.matmul(excl[:], lhsT=bstrict, rhs=csT_sb[:], start=True, stop=True)
    excl_sb = sb_pool.tile([128, 1], f32, name="scan_b_sb", tag="scan_b_sb")
    nc.vector.tensor_copy(excl_sb[:], excl[:])
    # transpose [128,1] -> [1,128]
    exclr = bps.tile([1, 128], f32, name="scan_c", tag="scan_c")
    nc.tensor.matmul(exclr[:], lhsT=excl_sb[:], rhs=ident, start=True, stop=True)
    exclr_sb = sb_pool.tile([1, 128], f32, name="scan_c_sb", tag="scan_c_sb")
    nc.vector.tensor_copy(exclr_sb[:], exclr[:])
    # broadcast the column offsets down all partitions
    nc.tensor.matmul(pos[:], lhsT=onesrow, rhs=exclr_sb[:], start=False, stop=True)
    return pos


def _emit(nc, capl, stage="full", use_bf16=True):
    nsl = E * capl
    nfull = capl // 128  # full 128-row c-tiles per expert group
    rem = capl % 128  # trailing partial c-tile rows (0 or 64)
    ctiles = [(i * 128, 128) for i in range(nfull)]
    if rem:
        ctiles.append((nfull * 128, rem))
    nct = len(ctiles)
    gdt = bf16 if use_bf16 else f32

    def _dump(tc, pool, dram_src, width, dt=f32):
        # copy dram_src[0:TS, 0:width] -> out rows via SBUF
        for i in range(NTT):
            t = pool.tile([128, width], dt, name="dumpt", tag="dumpt")
            nc.sync.dma_start(t[:], dram_src[i * 128 : (i + 1) * 128, 0:width])
            to = pool.tile([128, width], f32, name="dumpto", tag="dumpto")
            nc.vector.tensor_copy(to[:], t[:])
            nc.sync.dma_start(out[i * 128 : (i + 1) * 128, 0:width], to[:])

    xs = nc.dram_tensor("xs", [TS, H], f32, kind="ExternalInput").ap()
    wg = nc.dram_tensor("wg", [H, E], f32, kind="ExternalInput").ap()
    # host-prepacked weights: one contiguous DMA per (e, fg) / (e, hb) panel
    w1 = nc.dram_tensor("w1", [E, NFG, 128, HT * FG * 128], gdt, kind="ExternalInput").ap()
    w3 = nc.dram_tensor("w3", [E, NFG, 128, HT * FG * 128], gdt, kind="ExternalInput").ap()
    w2 = nc.dram_tensor("w2", [E, 2, 128, FT * 512], gdt, kind="ExternalInput").ap()
    cst = nc.dram_tensor("cst", [128, CSTW], f32, kind="ExternalInput").ap()
    premask = nc.dram_tensor("premask", [128, 1], f32, kind="ExternalInput").ap()
    out = nc.dram_tensor("out", [TS, H], f32, kind="ExternalOutput").ap()

    rloc = nc.dram_tensor("rloc", [TS, 2], f32).ap()
    # two-half AllGather outputs: half h holds all cores' token tiles
    # [h*8, h*8+8) in (core, tile, token) order
    rallA = nc.dram_tensor("rallA", [T // 2, 2], f32, addr_space="Shared").ap()
    rallB = nc.dram_tensor("rallB", [T // 2, 2], f32, addr_space="Shared").ap()
    xin = nc.dram_tensor("xin", [nsl, H], gdt).ap()
    eout = nc.dram_tensor("eout", [nsl, H], gdt).ap()

    with tile.TileContext(nc, num_cores=NCORES) as tc:
        with (
            tc.tile_pool(name="persist", bufs=1) as pp,
            tc.tile_pool(name="small", bufs=2) as sp,
        ):
            # ---- constants / static loads ----
            cst_sb = pp.tile([128, CSTW], f32, name="cst", tag="cst")
            nc.sync.dma_start(cst_sb[:], cst)
            ident = cst_sb[:, 0:128]
            ustrict = cst_sb[:, 128:256]
            onescol = cst_sb[:, 256:257]
            onesrow = cst_sb[0:1, 256:384]
            iota8 = cst_sb[:, 384:392]
            bstrict = cst_sb[:, 392:520]

            wg_sb = pp.tile([128, HT * E], f32, name="wg", tag="wg")
            nc.sync.dma_start(
                wg_sb[:].rearrange("p (n e) -> p n e", e=E),
                wg.rearrange("(n p) e -> p n e", p=128),
            )
            pm_sb = pp.tile([128, 1], f32, name="premask", tag="premask")
            nc.sync.dma_start(pm_sb[:], premask)

            # zero only the dummy landing block: slot 0 is the target of all
            # dropped/invalid gather indices, so row 0 must be finite.
            zsb = pp.tile([128, H], gdt, name="zeros", tag="zeros")
            nc.vector.memset(zsb[:], 0.0)
            nc.sync.dma_start(xin[0:128, :], zsb[:])

            ident_g = ident
            if use_bf16:
                identg_sb = pp.tile([128, 128], gdt, name="identg", tag="identg")
                nc.vector.tensor_copy(identg_sb[:], ident)
                ident_g = identg_sb[:]

            # persistent bookkeeping tiles
            jloc = pp.tile([128, NTT * E], f32, name="jloc", tag="jloc")
            oh1 = pp.tile([128, NTT * E], f32, name="oh1", tag="oh1")
            oh2 = pp.tile([128, NTT * E], f32, name="oh2", tag="oh2")
            i1f = pp.tile([128, NTT], f32, name="i1f", tag="i1f")
            i2f = pp.tile([128, NTT], f32, name="i2f", tag="i2f")
            w1l = pp.tile([128, NTT], f32, name="w1l", tag="w1l")
            w2l = pp.tile([128, NTT], f32, name="w2l", tag="w2l")
            vall = pp.tile([128, E * NTT], f32, name="vall", tag="vall")
            lrall = pp.tile([128, E * NTT], f32, name="lrall", tag="lrall")
            offb = pp.tile([128, E], f32, name="offb", tag="offb")
            idxb = pp.tile([128, NTT * 2], f32, name="idxb", tag="idxb")
            idxb3 = idxb[:].rearrange("p (n f) -> p n f", f=2)
            vbb = pp.tile([128, NTT * 2], f32, name="vbb", tag="vbb")
            vbb3 = vbb[:].rearrange("p (n f) -> p n f", f=2)
            sloti = [pp.tile([128, NTT], i32, name=f"slot{k}", tag=f"slot{k}") for k in range(2)]
            gidxi = [pp.tile([128, NTT], i32, name=f"gidx{k}", tag=f"gidx{k}") for k in range(2)]
            wp = [pp.tile([128, NTT], f32, name=f"wp{k}", tag=f"wp{k}") for k in range(2)]

            with tc.tile_pool(name="xspool", bufs=1) as xsp:
                xs_sb = xsp.tile([128, NTT * H], f32, name="xs", tag="xs")
                xs3 = xs_sb[:].rearrange("p (n h) -> p n h", h=H)
                if use_bf16:
                    xsg_sb = xsp.tile([128, NTT * H], gdt, name="xsg", tag="xsg")
                    xsg3 = xsg_sb[:].rearrange("p (n h) -> p n h", h=H)
                else:
                    xsg3 = xs3

                # ================= phase 1: gating =================
                with (
                    tc.tile_pool(name="gps", bufs=4, space="PSUM") as gps,
                    tc.tile_pool(name="gsb", bufs=4) as gsb,
                ):
                    for tt in range(NTT):
                        nc.sync.dma_start(
                            xs3[:, tt, :], xs[tt * 128 : (tt + 1) * 128, :]
                        )
                        if use_bf16:
                            # cast on the otherwise-idle scalar engine
                            nc.scalar.activation(
                                xsg3[:, tt, :], xs3[:, tt, :], AF.Copy
                            )
                        lg = gps.tile([128, E], f32, name="logits", tag="logits")
                        for hg in range(0, HT, 4):
                            # 4 transposes share one PSUM bank tile -> one
                            # wide copy, coarse PE<->DVE ping-pong
                            tpg = gps.tile([128, 512], f32, name="tpg", tag="tpg")
                            for j in range(4):
                                h = hg + j
                                nc.tensor.transpose(
                                    tpg[:, j * 128 : (j + 1) * 128],
                                    xs3[:, tt, h * 128 : (h + 1) * 128],
                                    ident,
                                )
                            xtg = gsb.tile([128, 512], f32, name="xtg", tag="xtg")
                            nc.vector.tensor_copy(xtg[:], tpg[:])
                            for j in range(4):
                                h = hg + j
                                nc.tensor.matmul(
                                    lg[:],
                                    lhsT=xtg[:, j * 128 : (j + 1) * 128],
                                    rhs=wg_sb[:].rearrange("p (n e) -> p n e", e=E)[
                                        :, h, :
                                    ],
                                    start=(h == 0),
                                    stop=(h == HT - 1),
                                )
                        lgs = gsb.tile([128, E], f32, name="lgs", tag="lgs")
                        nc.vector.tensor_copy(lgs[:], lg[:])
                        v8 = gsb.tile([128, 8], f32, name="v8", tag="v8")
                        nc.vector.max(out=v8[:], in_=lgs[:])
                        i8 = gsb.tile([128, 8], u32, name="i8", tag="i8")
                        nc.vector.max_index(out=i8[:], in_max=v8[:], in_values=lgs[:])
                        nc.vector.tensor_copy(idxb3[:, tt, :], i8[:, 0:2])
                        nc.vector.tensor_copy(vbb3[:, tt, :], v8[:, 0:2])
                        nc.sync.dma_start(
                            rloc[tt * 128 : (tt + 1) * 128, :], idxb3[:, tt, :]
                        )
                        if tt == NTT // 2 - 1:
                            # first-half routing is final: allgather it while
                            # the second half of gating runs
                            nc.gpsimd.collective_compute(
                                "AllGather",
                                OP.bypass,
                                replica_groups=[list(range(NCORES))],
                                ins=[rloc[0 : TS // 2, :]],
                                outs=[rallA],
                            )
                    # batched top-2 softmax over all tiles: w1 = 1/(1+d),
                    # w2 = d/(1+d) with d = exp(v2 - v1)
                    dd = gsb.tile([128, NTT], f32, name="dd", tag="dd")
                    nc.vector.tensor_tensor(
                        out=dd[:], in0=vbb3[:, :, 1], in1=vbb3[:, :, 0],
                        op=OP.subtract,
                    )
                    nc.scalar.activation(dd[:], dd[:], AF.Exp)
                    dp1 = gsb.tile([128, NTT], f32, name="dp1", tag="dp1")
                    nc.vector.tensor_scalar_add(dp1[:], dd[:], 1.0)
                    nc.vector.reciprocal(w1l[:], dp1[:])
                    nc.vector.tensor_tensor(
                        out=w2l[:], in0=dd[:], in1=w1l[:], op=OP.mult
                    )

                if stage == "gating":
                    with tc.tile_pool(name="dmp", bufs=2) as dmp:
                        _dump(tc, dmp, rloc, 2)
                    return

                # ============ phase 2+3: allgather + routing matrices ========
                with (
                    tc.tile_pool(name="bps", bufs=1, space="PSUM") as bps,
                    tc.tile_pool(name="ssb", bufs=3) as ssb,
                ):
                    # local routing blocks -> jloc/oh1/oh2/i tiles and the
                    # local scan; all of it reads only this core's routing, so
                    # it is emitted before the collective and overlaps it
                    jloc3 = jloc[:].rearrange("p (n e) -> p n e", e=E)
                    oh13 = oh1[:].rearrange("p (n e) -> p n e", e=E)
                    oh23 = oh2[:].rearrange("p (n e) -> p n e", e=E)
                    nc.vector.tensor_tensor(
                        out=oh13,
                        in0=idxb3[:, :, 0:1].broadcast_to([128, NTT, 8]),
                        in1=iota8.unsqueeze(1).broadcast_to([128, NTT, 8]),
                        op=OP.is_equal,
                    )
                    nc.vector.tensor_tensor(
                        out=oh23,
                        in0=idxb3[:, :, 1:2].broadcast_to([128, NTT, 8]),
                        in1=iota8.unsqueeze(1).broadcast_to([128, NTT, 8]),
                        op=OP.is_equal,
                    )
                    nc.vector.tensor_copy(i1f[:], idxb3[:, :, 0])
                    nc.vector.tensor_copy(i2f[:], idxb3[:, :, 1])
                    nc.vector.tensor_tensor(
                        out=jloc[:], in0=oh1[:], in1=oh2[:], op=OP.add
                    )
                    # batched local scan over all experts (position within
                    # this core's tokens, (n, e) column layout)
                    pos = _scan_all(
                        nc, bps, ssb, jloc[:], ident, ustrict, bstrict,
                        onescol, onesrow,
                    )
                    pos_sb = ssb.tile([128, NTT * E], f32, name="pos_sb", tag="pos_sb")
                    nc.vector.tensor_copy(pos_sb[:], pos[:])

                    nc.gpsimd.collective_compute(
                        "AllGather",
                        OP.bypass,
                        replica_groups=[list(range(NCORES))],
                        ins=[rloc[TS // 2 : TS, :]],
                        outs=[rallB],
                    )

                    # global per-(tile, expert) membership matrix [128, 128*8].
                    # Column groups are ordered (half, core, tile-within-half)
                    # so each AllGather half lands contiguously; the premask
                    # (earlier-core weighting) uses the same ordering.
                    jm = pp.tile([128, NGT * E], f32, name="jm", tag="jm")
                    jm3 = jm[:].rearrange("p (n e) -> p n e", e=E)
                    rsb = ssb.tile([128, NGT * 2], f32, name="rsb", tag="rsb")
                    rsb3 = rsb[:].rearrange("p (n f) -> p n f", f=2)
                    ohb = ssb.tile([128, NGT * E], f32, name="ohb", tag="ohb")
                    ohb3 = ohb[:].rearrange("p (n e) -> p n e", e=E)
                    # per-expert base offsets (count of earlier-core tokens)
                    # accumulate over the two AllGather halves; the A-half
                    # one-hots and count matmuls depend only on rallA, so they
                    # run underneath the second collective. Both halves share
                    # the same earlier-core mask on partitions 0:64 (the core
                    # index is (n2 % 64) // 8 in this column ordering).
                    HNG = NGT // 2
                    offr = bps.tile([1, E], f32, name="offrow", tag="offrow")
                    for hf, rsrc in ((0, rallA), (1, rallB)):
                        nsl0 = hf * HNG
                        nc.sync.dma_start(
                            rsb3[:, nsl0 : nsl0 + HNG, :],
                            rsrc.rearrange("(g p) f -> p g f", p=128),
                        )
                        nc.vector.tensor_tensor(
                            out=jm3[:, nsl0 : nsl0 + HNG, :],
                            in0=rsb3[:, nsl0 : nsl0 + HNG, 0:1]
                            .broadcast_to([128, HNG, 8]),
                            in1=iota8.unsqueeze(1).broadcast_to([128, HNG, 8]),
                            op=OP.is_equal,
                        )
                        nc.vector.tensor_tensor(
                            out=ohb3[:, nsl0 : nsl0 + HNG, :],
                            in0=rsb3[:, nsl0 : nsl0 + HNG, 1:2]
                            .broadcast_to([128, HNG, 8]),
                            in1=iota8.unsqueeze(1).broadcast_to([128, HNG, 8]),
                            op=OP.is_equal,
                        )
                        nc.vector.tensor_tensor(
                            out=jm3[:, nsl0 : nsl0 + HNG, :],
                            in0=jm3[:, nsl0 : nsl0 + HNG, :],
                            in1=ohb3[:, nsl0 : nsl0 + HNG, :],
                            op=OP.add,
                        )
                        for e in range(E):
                            csall = bps.tile([64, 1], f32, name="csall", tag="csall", bufs=2)
                            nc.tensor.matmul(
                                csall[:],
                                lhsT=jm3[:, nsl0 : nsl0 + HNG, e],
                                rhs=onescol,
                                start=True,
                                stop=True,
                            )
                            csall_sb = ssb.tile([64, 1], f32, name="csall_sb", tag="csall_sb")
                            nc.vector.tensor_copy(csall_sb[:], csall[:])
                            nc.tensor.matmul(
                                offr[0:1, e : e + 1],
                                lhsT=csall_sb[:],
                                rhs=pm_sb[0:64, :],
                                start=(hf == 0),
                                stop=(hf == 1),
                            )
                    offr_sb = ssb.tile([1, E], f32, name="offr_sb", tag="offr_sb")
                    nc.vector.tensor_copy(offr_sb[:], offr[:])
                    offbp = bps.tile([128, E], f32, name="offbp", tag="offbp")
                    nc.tensor.matmul(
                        offbp[:], lhsT=onesrow, rhs=offr_sb[:], start=True, stop=True
                    )
                    nc.vector.tensor_copy(offb[:], offbp[:])

                    # global position -> validity -> local rank
                    gpos = ssb.tile([128, NTT * E], f32, name="gpos", tag="gpos")
                    nc.vector.tensor_tensor(
                        out=gpos[:].rearrange("p (n e) -> p n e", e=E),
                        in0=pos_sb[:].rearrange("p (n e) -> p n e", e=E),
                        in1=offb[:].unsqueeze(1).broadcast_to([128, NTT, E]),
                        op=OP.add,
                    )
                    nc.vector.scalar_tensor_tensor(
                        out=vall[:],
                        in0=gpos[:],
                        scalar=float(CAP),
                        in1=jloc[:],
                        op0=OP.is_lt,
                        op1=OP.mult,
                    )
                    lr = _scan_all(
                        nc, bps, ssb, vall[:], ident, ustrict, bstrict,
                        onescol, onesrow,
                    )
                    nc.vector.tensor_copy(lrall[:], lr[:])

                    # ---- per-assignment slot / gather-index / weight ----
                    for k, (ikf, ohk, wkl) in enumerate(
                        [(i1f, oh1, w1l), (i2f, oh2, w2l)]
                    ):
                        lrp = ssb.tile([128, NTT], f32, name=f"lrp{k}", tag=f"lrp{k}")
                        vpk = ssb.tile([128, NTT], f32, name=f"vpk{k}", tag=f"vpk{k}")
                        tmp = ssb.tile([128, NTT], f32, name=f"tmp{k}", tag=f"tmp{k}")
                        t128 = ssb.tile([128, NTT * E], f32, name=f"t128_{k}", tag=f"t128_{k}")
                        nc.vector.tensor_tensor(
                            out=t128[:], in0=ohk[:], in1=lrall[:], op=OP.mult
                        )
                        nc.vector.tensor_reduce(
                            out=lrp[:],
                            in_=t128[:].rearrange("p (n e) -> p n e", e=E),
                            axis=mybir.AxisListType.X,
                            op=OP.add,
                        )
                        nc.vector.tensor_tensor(
                            out=t128[:], in0=ohk[:], in1=vall[:], op=OP.mult
                        )
                        nc.vector.tensor_reduce(
                            out=vpk[:],
                            in_=t128[:].rearrange("p (n e) -> p n e", e=E),
                            axis=mybir.AxisListType.X,
                            op=OP.add,
                        )
                        # slot = e*capl + lrank, or >= nsl when invalid
                        slot = ssb.tile([128, NTT], f32, name=f"slotf{k}", tag=f"slotf{k}")
                        nc.vector.scalar_tensor_tensor(
                            out=slot[:],
                            in0=ikf[:],
                            scalar=float(capl),
                            in1=lrp[:],
                            op0=OP.mult,
                            op1=OP.add,
                        )
                        nc.vector.tensor_scalar(
                            out=tmp[:],
                            in0=vpk[:],
                            scalar1=0.5,
                            scalar2=None,
                            op0=OP.is_lt,
                        )
                        nc.vector.scalar_tensor_tensor(
                            out=slot[:],
                            in0=tmp[:],
                            scalar=BIG,
                            in1=slot[:],
                            op0=OP.mult,
                            op1=OP.add,
                        )
                        nc.vector.tensor_copy(sloti[k][:], slot[:])
                        # gather idx = clamped slot, 0 when invalid
                        nc.vector.tensor_scalar_min(tmp[:], lrp[:], float(capl - 1))
                        nc.vector.scalar_tensor_tensor(
                            out=tmp[:],
                            in0=ikf[:],
                            scalar=float(capl),
                            in1=tmp[:],
                            op0=OP.mult,
                            op1=OP.add,
                        )
                        nc.vector.tensor_tensor(
                            out=tmp[:], in0=tmp[:], in1=vpk[:], op=OP.mult
                        )
                        nc.vector.tensor_copy(gidxi[k][:], tmp[:])
                        # combine weight = w_k * valid
                        nc.vector.tensor_tensor(
                            out=wp[k][:], in0=wkl[:], in1=vpk[:], op=OP.mult
                        )

                if stage == "scans":
                    with tc.tile_pool(name="dmp", bufs=2) as dmp:
                        t = dmp.tile([128, 96], f32, name="dumps", tag="dumps")
                        for j, src in enumerate([sloti[0], sloti[1], gidxi[0], gidxi[1], wp[0], wp[1]]):
                            nc.vector.tensor_copy(t[:, j * 16 : (j + 1) * 16], src[:])
                        nc.sync.dma_start(out[0:128, 0:96], t[:])
                    return

                # ============ phase 5: scatter token rows into groups ============
                # The indirect DMA consumes exactly one offset per partition,
                # so this is one call per 128-token tile. Slots are unique per
                # assignment, so the calls write disjoint rows; a critical
                # section with manual completion semaphores lets them stream
                # back-to-back on the gpsimd queue instead of paying a
                # completion round-trip between each pair.
                if os.environ.get("MOE_SCRIT", "1") not in ("", "0"):
                    ssem = nc.alloc_semaphore("scat_sem")
                    with tc.tile_critical():
                        for k in range(2):
                            for tt in range(NTT):
                                nc.gpsimd.indirect_dma_start(
                                    out=xin,
                                    out_offset=IndirectOffsetOnAxis(
                                        ap=sloti[k][:, tt : tt + 1], axis=0
                                    ),
                                    in_=xsg3[:, tt, :],
                                    in_offset=None,
                                    bounds_check=nsl - 1,
                                    oob_is_err=False,
                                ).then_inc(ssem, 16)
                        nc.gpsimd.nop(nofuse=True, hint="scat_wait")._wait_ge(
                            ssem, 2 * NTT * 16
                        )
                else:
                    for k in range(2):
                        for tt in range(NTT):
                            nc.gpsimd.indirect_dma_start(
                                out=xin,
                                out_offset=IndirectOffsetOnAxis(
                                    ap=sloti[k][:, tt : tt + 1], axis=0
                                ),
                                in_=xsg3[:, tt, :],
                                in_offset=None,
                                bounds_check=nsl - 1,
                                oob_is_err=False,
                            )

            if stage == "scatter":
                with tc.tile_pool(name="dmp", bufs=2) as dmp:
                    _dump(tc, dmp, xin, H, dt=gdt)
                return

            # ================= phase 6: expert FFNs =================
            with (
                tc.tile_pool(name="fps_tp", bufs=2, space="PSUM") as fps_tp,
                tc.tile_pool(name="fps_gu", bufs=2, space="PSUM") as fps_gu,
                tc.tile_pool(name="fps_e", bufs=2, space="PSUM") as fps_e,
                tc.tile_pool(name="fsb", bufs=1) as fsb,
                tc.tile_pool(name="fw", bufs=3) as fw,
                tc.tile_pool(name="fio", bufs=2) as fio,
            ):
                cc = [(0, min(512, capl))]
                if capl > 512:
                    cc.append((512, capl - 512))

                def build_actT(e):
                    # build transposed activations on the PE (identity matmuls)
                    actT = fsb.tile([128, HT * capl], gdt, name="actT", tag="actT", bufs=3)
                    actT3 = actT[:].rearrange("p (h c) -> p h c", c=capl)
                    for r0c, rws in ctiles:
                        r0 = e * capl + r0c
                        xi = fio.tile([128, H], gdt, name="xin_sb", tag="xin_sb", bufs=3)
                        nc.sync.dma_start(xi[0:rws, :], xin[r0 : r0 + rws, :])
                        for h in range(HT):
                            tp = fps_tp.tile([128, 128], gdt, name="ffn_tp", tag="ffn_tp")
                            nc.tensor.transpose(
                                tp[:, 0:rws],
                                xi[0:rws, h * 128 : (h + 1) * 128],
                                ident_g[0:rws, 0:rws],
                            )
                            nc.vector.tensor_copy(
                                actT3[:, h, r0c : r0c + rws], tp[:, 0:rws]
                            )
                    return actT3

                actT3_cur = build_actT(0)
                for e in range(E):
                    actT3 = actT3_cur
                    hT = fsb.tile([128, FT * capl], gdt, name="hT", tag="hT", bufs=2)
                    hT3 = hT[:].rearrange("p (f c) -> p f c", c=capl)
                    for fg0 in range(0, FT, FG):
                        fgi = fg0 // FG
                        w1g = fw.tile([128, HT * FG * 128], gdt, name="w1g", tag="w1g")
                        w3g = fw.tile([128, HT * FG * 128], gdt, name="w3g", tag="w3g")
                        w1g3 = w1g[:].rearrange("p (h f) -> p h f", f=FG * 128)
                        w3g3 = w3g[:].rearrange("p (h f) -> p h f", f=FG * 128)
                        nc.sync.dma_start(w1g[:], w1[e, fgi])
                        nc.sync.dma_start(w3g[:], w3[e, fgi])
                        for ft in range(fg0, fg0 + FG):
                            fo = (ft - fg0) * 128
                            ga = [
                                fps_gu.tile([128, w_], f32, name=f"gu{ci}", tag=f"gu{ci}")
                                for ci, (_, w_) in enumerate(cc)
                            ]
                            for h in range(HT):
                                for ci, (c0, w_) in enumerate(cc):
                                    nc.tensor.matmul(
                                        ga[ci][:],
                                        lhsT=w1g3[:, h, fo : fo + 128],
                                        rhs=actT3[:, h, c0 : c0 + w_],
                                        start=(h == 0),
                                        stop=(h == HT - 1),
                                    )
                            # t = silu(g) = g * sigmoid(g)
                            tsl = fio.tile([128, capl], f32, name="tsilu", tag="tsilu")
                            for ci, (c0, w_) in enumerate(cc):
                                nc.scalar.activation(
                                    tsl[:, c0 : c0 + w_], ga[ci][:], AF.Sigmoid
                                )
                                nc.vector.tensor_tensor(
                                    out=tsl[:, c0 : c0 + w_],
                                    in0=tsl[:, c0 : c0 + w_],
                                    in1=ga[ci][:],
                                    op=OP.mult,
                                )
                            # u = x @ W3 (reuse psum slots)
                            ua = [
                                fps_gu.tile([128, w_], f32, name=f"gu{ci}", tag=f"gu{ci}")
                                for ci, (_, w_) in enumerate(cc)
                            ]
                            for h in range(HT):
                                for ci, (c0, w_) in enumerate(cc):
                                    nc.tensor.matmul(
                                        ua[ci][:],
                                        lhsT=w3g3[:, h, fo : fo + 128],
                                        rhs=actT3[:, h, c0 : c0 + w_],
                                        start=(h == 0),
                                        stop=(h == HT - 1),
                                    )
                            # hT = silu(g) * u
                            for ci, (c0, w_) in enumerate(cc):
                                nc.vector.tensor_tensor(
                                    out=hT3[:, ft, c0 : c0 + w_],
                                    in0=tsl[:, c0 : c0 + w_],
                                    in1=ua[ci][:],
                                    op=OP.mult,
                                )
                    # emit the next expert's activation-transpose build here so
                    # its DVE copies drain underneath pass 2's matmul stream
                    if e + 1 < E:
                        actT3_cur = build_actT(e + 1)
                    # pass 2: eout = hT.T @ W2
                    for hb in range(2):
                        w2r = fsb.tile([128, FT * 512], gdt, name="w2row", tag="w2row", bufs=2)
                        w2r3 = w2r[:].rearrange("p (f x) -> p f x", x=512)
                        nc.sync.dma_start(w2r[:], w2[e, hb])
                        eo = fio.tile([128, nct * 512], gdt, name="eo_sb", tag="eo_sb")
                        eo3 = eo[:].rearrange("p (n x) -> p n x", x=512)
                        for ci, (r0c, rws) in enumerate(ctiles):
                            eps = fps_e.tile([128, 512], f32, name="eps", tag="eps")
                            for ft in range(FT):
                                nc.tensor.matmul(
                                    eps[0:rws, :],
                                    lhsT=hT3[:, ft, r0c : r0c + rws],
                                    rhs=w2r3[:, ft, :],
                                    start=(ft == 0),
                                    stop=(ft == FT - 1),
                                )
                            nc.vector.tensor_copy(eo3[0:rws, ci, :], eps[0:rws, :])
                        # batched store per (e, hb): full tiles in one
                        # rearranged DMA, trailing partial tile separately
                        nc.sync.dma_start(
                            eout[e * capl : e * capl + nfull * 128,
                                 hb * 512 : (hb + 1) * 512]
                            .rearrange("(n p) x -> p n x", p=128),
                            eo3[:, 0:nfull, :],
                        )
                        if rem:
                            nc.sync.dma_start(
                                eout[e * capl + nfull * 128 : (e + 1) * capl,
                                     hb * 512 : (hb + 1) * 512],
                                eo3[0:rem, nfull, :],
                            )

                if stage == "ffn":
                    with tc.tile_pool(name="dmp2", bufs=2) as dmp2:
                        _dump(tc, dmp2, eout, H, dt=gdt)
                    return

                # ================= phase 7: combine =================
                for tt in range(NTT):
                    r1 = fio.tile([128, H], gdt, name="r1", tag="r1")
                    nc.gpsimd.indirect_dma_start(
                        out=r1[:],
                        out_offset=None,
                        in_=eout,
                        in_offset=IndirectOffsetOnAxis(
                            ap=gidxi[0][:, tt : tt + 1], axis=0
                        ),
                    )
                    r2 = fio.tile([128, H], gdt, name="r2", tag="r2")
                    nc.gpsimd.indirect_dma_start(
                        out=r2[:],
                        out_offset=None,
                        in_=eout,
                        in_offset=IndirectOffsetOnAxis(
                            ap=gidxi[1][:, tt : tt + 1], axis=0
                        ),
                    )
                    ot = fio.tile([128, H], f32, name="ot", tag="ot")
                    nc.vector.tensor_scalar(
                        out=ot[:],
                        in0=r1[:],
                        scalar1=wp[0][:, tt : tt + 1],
                        scalar2=None,
                        op0=OP.mult,
                    )
                    nc.vector.scalar_tensor_tensor(
                        out=ot[:],
                        in0=r2[:],
                        scalar=wp[1][:, tt : tt + 1],
                        in1=ot[:],
                        op0=OP.mult,
                        op1=OP.add,
                    )
                    nc.sync.dma_start(out[tt * 128 : (tt + 1) * 128, :], ot[:])

    return nc


_LDW_PATCHED = False


def _enable_ldw_opt():
    """Swap the hardcoded --enable-ldw-opt=false walrus flag to true: every
    fp32 matmul otherwise pays an unoverlapped LDWEIGHTS (~40% PE time)."""
    global _LDW_PATCHED
    if _LDW_PATCHED:
        return
    from concourse import bass_utils as _bu

    _orig = _bu.run_command

    def _patched(argv, **kw):
        argv = [
            a.replace("--enable-ldw-opt=false", "--enable-ldw-opt=true")
            if isinstance(a, str)
            else a
            for a in argv
        ]
        return _orig(argv, **kw)

    _bu.run_command = _patched
    _LDW_PATCHED = True


_NC_CACHE = {}


def _get_nc(capl, stage="full", use_bf16=True):
    # NOTE: --enable-ldw-opt=true crashes walrus codegen (visitInstLdweights,
    # CoreV3GenImpl.cpp:694) on this kernel — keep it off.
    if os.environ.get("MOE_LDW_OPT", "0") not in ("", "0"):
        _enable_ldw_opt()
    key = (capl, stage, use_bf16)
    if key not in _NC_CACHE:
        nc = bacc.Bacc("TRN2", debug=False, num_devices=NCORES)
        _emit(nc, capl, stage, use_bf16)
        nc.compile()
        _NC_CACHE[key] = nc
    return _NC_CACHE[key]


def _host_max_local_count(x, Wg):
    """Cheap host routing replica: max kept-assignments per (core, expert)."""
    logits = x.astype(np.float32) @ Wg.astype(np.float32)
    i1 = np.argmax(logits, axis=1)
    m = logits.copy()
    m[np.arange(T), i1] = -np.inf
    i2 = np.argmax(m, axis=1)
    routed = np.zeros((T, E), dtype=np.int64)
    routed[np.arange(T), i1] = 1
    routed[np.arange(T), i2] += 1
    pos = np.cumsum(routed, axis=0) - routed
    keep = routed * (pos < CAP)
    counts = keep.reshape(NCORES, TS, E).sum(axis=1)
    return int(counts.max())


def _install_ntff_hook():
    """Best-effort registration of the axon NTFF profiling hook (for tracing)."""
    import sys
    import types

    if "antenv.axon_hooks" in sys.modules:
        return
    try:
        mod = types.ModuleType("antenv.axon_hooks")
        hook = [None]
        mod.set_axon_ntff_profile_hook = lambda h: hook.__setitem__(0, h)
        mod.get_axon_ntff_profile_hook = lambda: hook[0]
        from trn_agent_boot.trn_boot import _ntff_profile_via_ctypes

        mod.set_axon_ntff_profile_hook(
            _ntff_profile_via_ctypes("/opt/axon/libaxon_pjrt.so")
        )
        sys.modules["antenv.axon_hooks"] = mod
    except Exception:
        pass


def kernel(x, Wg, W1, W3, W2):
    global LAST_RESULTS
    x = np.ascontiguousarray(np.asarray(x, dtype=np.float32))
    Wg = np.ascontiguousarray(np.asarray(Wg, dtype=np.float32))
    W1 = np.asarray(W1, dtype=np.float32)
    W3 = np.asarray(W3, dtype=np.float32)
    W2 = np.asarray(W2, dtype=np.float32)

    # static per-(core, expert) group capacity with safety margin (device
    # routing could diverge from this host estimate only on exact ties)
    maxc = _host_max_local_count(x, Wg)
    capl = max(128, int(math.ceil((maxc + 16) / 64.0) * 64))

    use_bf16 = os.environ.get("MOE_GDT", "bf16") != "f32"
    nc = _get_nc(capl, os.environ.get("MOE_STAGE", "full"), use_bf16)
    cst = _build_consts()
    wdt = np.float32
    if use_bf16:
        import ml_dtypes

        wdt = ml_dtypes.bfloat16
    # host-prepack the weight panels so each on-device load is one
    # contiguous-per-partition DMA:
    #   w1p[e, fg, p, (ht, fo)] = W1[e, ht*128+p, fg*256+fo]
    #   w2p[e, hb, p, (ft, x)]  = W2[e, ft*128+p, hb*512+x]
    w1p = np.ascontiguousarray(
        W1.reshape(E, HT, 128, NFG, FG * 128).transpose(0, 3, 2, 1, 4)
        .reshape(E, NFG, 128, HT * FG * 128).astype(wdt)
    )
    w3p = np.ascontiguousarray(
        W3.reshape(E, HT, 128, NFG, FG * 128).transpose(0, 3, 2, 1, 4)
        .reshape(E, NFG, 128, HT * FG * 128).astype(wdt)
    )
    w2p = np.ascontiguousarray(
        W2.reshape(E, FT, 128, 2, 512).transpose(0, 3, 2, 1, 4)
        .reshape(E, 2, 128, FT * 512).astype(wdt)
    )
    in_maps = []
    for c in range(NCORES):
        # premask indexing matches jm's (half, core, tile) column order:
        # entry p belongs to core (p % 64) // 8
        pm = ((np.arange(128) % 64) // 8 < c).astype(np.float32)[:, None]
        in_maps.append(
            {
                "xs": x[c * TS : (c + 1) * TS],
                "wg": Wg,
                "w1": w1p,
                "w3": w3p,
                "w2": w2p,
                "cst": cst,
                "premask": np.ascontiguousarray(pm),
            }
        )

    trace = os.environ.get("BASS_TRACE", "") not in ("", "0", "false", "False")
    if trace:
        _install_ntff_hook()
    res = run_bass_kernel_spmd(nc, in_maps, list(range(NCORES)), trace=trace)
    LAST_RESULTS = res
    return np.concatenate([res.results[c]["out"] for c in range(NCORES)], axis=0)
